# revision 38
# baseline (speedup 1.0000x reference)
"""Trainium2 Bass kernel for nn_CM_NTM_29566554866014 (scatter_memory).

Sharding: pure batch data-parallelism across 8 NeuronCores (B=2048 -> 256/core).
Small parameters replicated. The cross-NTM loop (T=4) is sequential but
batch-local, so each core runs all 4 steps on its batch shard independently.
No collectives.

Key structural facts used (verified against the reference math):
  * The write head (Ww/bw/ww0) and the memory erase/add update are dead code:
    `mem` is reassigned to `mem0[i+1]` each iteration and outputs depend only
    on h and r. They are therefore not computed.
  * Only read0[T-1] is consumed.
  * Per-step state (mem0/h0/c0/wr0) are fresh inputs each step; the only
    sequential dependency across steps is the read vector r.

Engine assignment (v5, evolved via perfetto traces; 524us -> ~426us):
  * DVE (vector) is the bottleneck engine; every big elementwise op is bf16
    so it runs in DVE 2x mode. Empirical 2x rule on TRN2: a stride-0
    broadcast on a MIDDLE free dim keeps 2x (cos numerator: k broadcast
    over n), a stride-0 broadcast on the INNERMOST dim drops to 1x (read
    weighting: w broadcast over m - unavoidable in this layout).
  * gpsimd is a trap here: it shares SBUF read/write ports with the DVE,
    so offloading big elementwise ops to it inflates concurrent DVE ops
    3-5x. All elementwise stays on DVE/ACT.
  * mem row-norm squares run on the ACT engine (Square), grouped with the
    LN squares into one table visit; activation calls are grouped per
    function to minimise ACT table reloads (1.28us each).
  * addressing math is Exp/Ln-only: 1/(|m||k|) and *beta folded into one
    Exp(-0.5*ln(n2*k2) + ln(beta)) ACT op; softmax max-subtraction dropped
    (logits bounded); softplus via shared Exp/Ln blocks; sharpen left
    unnormalised and the 1/sum folded into the read vector.
  * read-head output computed transposed on the PE (h as stationary), which
    removes the k/kh PE transposes + drains; bias via a ones-row matmul.
  * output projection of step t is deferred into step t+1 so the PE runs
    proj(t+1) while the DVE walks t's addressing chain.
  * tensor_tensor_reduce crashes the HW runtime (NRT_EXEC_UNIT_UNRECOVERABLE)
    - do not use it.
"""

import numpy as np
import ml_dtypes
from contextlib import ExitStack

import concourse.bass as bass
import concourse.tile as tile
from concourse import bacc
from concourse import mybir
from concourse.bass_utils import run_bass_kernel_spmd
from concourse.masks import make_identity

AF = mybir.ActivationFunctionType
ALU = mybir.AluOpType
AX = mybir.AxisListType
FP = mybir.dt.float32
BF = mybir.dt.bfloat16

T, E, V, H, N, M, B = 4, 512, 256, 512, 128, 64, 2048
NCORES = 8
BS = B // NCORES      # 256 batch rows per core
NBT = BS // 128       # 2 batch tiles
HC = H // 128         # 4
EC = E // 128         # 4
VC = V // 128         # 2
ZC = (4 * H) // 128   # 16


def _bcast_inner(ap, count):
    """View `ap` ([P, F]) as [P, F, count] with a stride-0 innermost dim."""
    return bass.AP(tensor=ap.tensor, offset=ap.offset,
                   ap=[*ap.ap, [0, count]])


def _bcast_mid(ap, count):
    """View `ap` ([P, F]) as [P, count, F] with a stride-0 middle dim."""
    return bass.AP(tensor=ap.tensor, offset=ap.offset,
                   ap=[ap.ap[0], [0, count], ap.ap[1]])


def _cols(ap, start, step, count):
    """Strided column view of a 2-dim AP: columns start, start+step, ..."""
    st = ap.ap[1][0]
    return bass.AP(tensor=ap.tensor, offset=ap.offset + start * st,
                   ap=[ap.ap[0], [step * st, count]])


def build_nc(stage=None):
    import os
    if stage is None:
        stage = int(os.environ.get("NTM_STAGE", "99"))
    nc = bacc.Bacc()
    d = {}

    def din(name, shape, dt=FP):
        d[name] = nc.dram_tensor(name, list(shape), dt, kind="ExternalInput")

    din("xT",   (T, E, BS), BF)
    din("w1t",  (T, E, H), BF)
    din("w2t",  (T, H, V), BF)
    din("wiht", (T, V + M, 4 * H), BF)
    din("whht", (T, H, 4 * H), BF)
    din("wrt",  (T, H, M + 6), BF)
    din("wot",  (T, H + M, E), BF)
    din("h0t",  (T, H, BS), BF)
    din("c0t",  (T, H, BS), BF)
    din("r0t",  (M, BS), BF)
    din("wr0",  (T, BS, N), BF)
    din("mem0", (T, BS, N, M), BF)
    din("b1c",  (T, 128, HC))
    din("lngc", (T, 128, HC))
    din("lnbc", (T, 128, HC))
    din("b2c",  (T, 128, VC))
    din("bzc",  (T, 128, ZC))
    din("brr",  (T, 1, M + 6))
    din("boc",  (T, 128, EC))
    outT = nc.dram_tensor("outT", [T, E, BS], FP, kind="ExternalOutput")

    with tile.TileContext(nc) as tc, ExitStack() as ctx:
        singles = ctx.enter_context(tc.tile_pool(name="singles", bufs=1))
        wpool = ctx.enter_context(tc.tile_pool(name="wpool", bufs=1))
        spool = ctx.enter_context(tc.tile_pool(name="spool", bufs=1))
        apool = ctx.enter_context(tc.tile_pool(name="apool", bufs=1))
        mpool = ctx.enter_context(tc.tile_pool(name="mpool", bufs=1))
        ppool = ctx.enter_context(tc.tile_pool(name="ppool", bufs=1))
        pmm = ctx.enter_context(tc.tile_pool(name="pmm", bufs=1, space="PSUM"))

        ones_t = singles.tile([128, 128], FP, name="ones_t")
        nc.vector.memset(ones_t, 1.0)
        ones_row = singles.tile([1, 128], FP, name="ones_row")
        nc.vector.memset(ones_row, 1.0)
        ones_b = singles.tile([128, 128], BF, name="ones_b")
        nc.vector.memset(ones_b, 1.0)
        ident = singles.tile([128, 128], FP, name="ident")
        make_identity(nc, ident)
        eps30 = singles.tile([128, 1], FP, name="eps30")
        nc.vector.memset(eps30, 1e-30)

        def mm_ps(shape, name, tag="mm", bufs=5):
            return pmm.tile(shape, FP, name=name, tag=tag, bufs=bufs)

        def tree_m(dst2d, prod, eng=None):
            """Sum prod [128, N, M] over innermost m into dst2d [128, N] fp32
            via pairwise bf16 adds (DVE 2x mode)."""
            eng = eng or nc.vector
            G = prod.shape[1]
            s1 = ppool.tile([128, G, M // 2], BF, name="trm", tag="trm", bufs=1)
            eng.tensor_add(s1, prod[:, :, 0:M // 2], prod[:, :, M // 2:M])
            w = M // 2
            while w > 2:
                hw = w // 2
                eng.tensor_add(s1[:, :, 0:hw], s1[:, :, 0:hw], s1[:, :, hw:w])
                w = hw
            dst3 = bass.AP(tensor=dst2d.tensor, offset=dst2d.offset,
                           ap=[*dst2d.ap, [1, 1]])
            eng.tensor_add(dst3, s1[:, :, 0:1], s1[:, :, 1:2])

        def tree_n(dst3d, prod):
            """Sum prod [128, N, M] over mid n into dst3d [128, 1, M] fp32
            via pairwise bf16 adds on contiguous halves."""
            G = prod.shape[1]
            s1 = ppool.tile([128, G // 2, M], BF, name="trn", tag="trn", bufs=1)
            nc.vector.tensor_add(s1, prod[:, 0:G // 2, :], prod[:, G // 2:G, :])
            w = G // 2
            while w > 2:
                hw = w // 2
                nc.vector.tensor_add(s1[:, 0:hw, :], s1[:, 0:hw, :],
                                     s1[:, hw:w, :])
                w = hw
            nc.vector.tensor_add(dst3d, s1[:, 0:1, :], s1[:, 1:2, :])

        mem = {}
        sqn = {}

        def load_mem(t):
            for bt in range(NBT):
                mt = mpool.tile([128, N, M], BF, name=f"mem_t{t}_{bt}",
                                tag="mem", bufs=4)
                nc.sync.dma_start(out=mt, in_=d["mem0"][t, bt * 128:(bt + 1) * 128])
                mem[(t, bt)] = mt

        def emit_out(to, wo_, h_, rT_, boc_):
            for ec in range(EC):
                esl = slice(ec * 128, (ec + 1) * 128)
                ps = mm_ps([128, BS], f"o_t{to}_{ec}")
                for k in range(4):
                    nc.tensor.matmul(ps, wo_[k][:, esl], h_[k], start=(k == 0),
                                     stop=False)
                nc.tensor.matmul(ps, wo_[4][:, esl], rT_, start=False,
                                 stop=True)
                os_ = apool.tile([128, BS], FP, name=f"os_t{to}_{ec}", tag="os",
                                 bufs=2)
                nc.scalar.activation(out=os_, in_=ps, func=AF.Sigmoid,
                                     bias=boc_[:, ec:ec + 1])
                nc.sync.dma_start(out=outT[to, esl, :], in_=os_)

        pending_out = None
        rT_prev = None
        for t in range(T):
            sfx = f"t{t}"
            # ---------------- loads ----------------
            # Issue order matters: the sync engine fires DMAs in program
            # order, so load what this t needs first (proj inputs, mem for
            # the ACT squares), then the late-phase weights, then prefetch.
            w1 = [wpool.tile([128, H], BF, name=f"w1_{sfx}_{k}", tag="w1",
                             bufs=4) for k in range(4)]
            for k in range(4):
                nc.sync.dma_start(out=w1[k], in_=d["w1t"][t, k * 128:(k + 1) * 128, :])
            xT = [spool.tile([128, BS], BF, name=f"xT_{sfx}_{k}", tag="xT",
                             bufs=4) for k in range(4)]
            for k in range(4):
                nc.sync.dma_start(out=xT[k], in_=d["xT"][t, k * 128:(k + 1) * 128, :])
            b1c = spool.tile([128, HC], FP, name=f"b1c_{sfx}", tag="b1c", bufs=2)
            lng = spool.tile([128, HC], FP, name=f"lng_{sfx}", tag="lng", bufs=2)
            lnb = spool.tile([128, HC], FP, name=f"lnb_{sfx}", tag="lnb", bufs=2)
            b2c = spool.tile([128, VC], FP, name=f"b2c_{sfx}", tag="b2c", bufs=2)
            bzc = spool.tile([128, ZC], FP, name=f"bzc_{sfx}", tag="bzc", bufs=2)
            brr = spool.tile([1, M + 6], FP, name=f"brr_{sfx}", tag="brr", bufs=2)
            boc = spool.tile([128, EC], FP, name=f"boc_{sfx}", tag="boc", bufs=2)
            nc.sync.dma_start(out=b1c, in_=d["b1c"][t])
            nc.sync.dma_start(out=lng, in_=d["lngc"][t])
            nc.sync.dma_start(out=lnb, in_=d["lnbc"][t])
            nc.sync.dma_start(out=b2c, in_=d["b2c"][t])
            if t == 0:
                load_mem(0)
            w2 = [wpool.tile([128, V], BF, name=f"w2_{sfx}_{k}", tag="w2",
                             bufs=4) for k in range(4)]
            for k in range(4):
                nc.sync.dma_start(out=w2[k], in_=d["w2t"][t, k * 128:(k + 1) * 128, :])
            h0 = [spool.tile([128, BS], BF, name=f"h0_{sfx}_{k}", tag="h0",
                             bufs=4) for k in range(4)]
            c0b = spool.tile([128, HC, BS], BF, name=f"c0b_{sfx}", tag="c0",
                             bufs=2)
            for k in range(4):
                nc.sync.dma_start(out=h0[k], in_=d["h0t"][t, k * 128:(k + 1) * 128, :])
                nc.sync.dma_start(out=c0b[:, k, :],
                                  in_=d["c0t"][t, k * 128:(k + 1) * 128, :])
            nc.sync.dma_start(out=bzc, in_=d["bzc"][t])
            wih = []
            for k, ksz in enumerate((128, 128, 64)):
                wt = wpool.tile([ksz, 4 * H], BF, name=f"wih_{sfx}_{k}", tag="wih",
                                bufs=3)
                nc.sync.dma_start(out=wt, in_=d["wiht"][t, k * 128:k * 128 + ksz, :])
                wih.append(wt)
            whh = [wpool.tile([128, 4 * H], BF, name=f"whh_{sfx}_{k}", tag="whh",
                              bufs=4) for k in range(4)]
            for k in range(4):
                nc.sync.dma_start(out=whh[k], in_=d["whht"][t, k * 128:(k + 1) * 128, :])
            wr_ = [wpool.tile([128, M + 6], BF, name=f"wr_{sfx}_{k}", tag="wr",
                              bufs=4) for k in range(4)]
            for k in range(4):
                nc.sync.dma_start(out=wr_[k], in_=d["wrt"][t, k * 128:(k + 1) * 128, :])
            nc.sync.dma_start(out=brr, in_=d["brr"][t])
            w0 = []
            for bt in range(NBT):
                wt = spool.tile([128, N], BF, name=f"w0_{sfx}_{bt}", tag="w0",
                                bufs=4)
                nc.sync.dma_start(out=wt, in_=d["wr0"][t, bt * 128:(bt + 1) * 128, :])
                w0.append(wt)
            wo = []
            for k, ksz in enumerate((128, 128, 128, 128, 64)):
                wt = wpool.tile([ksz, E], BF, name=f"wo_{sfx}_{k}", tag="wo", bufs=5)
                nc.sync.dma_start(out=wt, in_=d["wot"][t, k * 128:k * 128 + ksz, :])
                wo.append(wt)
            nc.sync.dma_start(out=boc, in_=d["boc"][t])
            if t + 1 < T:
                load_mem(t + 1)

            if t == 0:
                rT_prev = spool.tile([M, BS], BF, name="r0T", tag="rT", bufs=2)
                nc.sync.dma_start(out=rT_prev, in_=d["r0t"][:, :])

            # ---------------- input projection + LN + p ----------------
            a1 = apool.tile([128, HC, BS], FP, name=f"a1_{sfx}", tag="a1", bufs=1)
            for hc in range(HC):
                ps = mm_ps([128, BS], f"a1_{sfx}_{hc}")
                for k in range(4):
                    nc.tensor.matmul(ps, w1[k][:, hc * 128:(hc + 1) * 128], xT[k],
                                     start=(k == 0), stop=(k == 3))
                nc.vector.tensor_scalar(out=a1[:, hc, :], in0=ps,
                                        scalar1=b1c[:, hc:hc + 1], scalar2=None,
                                        op0=ALU.add)

            ps_sum = mm_ps([128, BS], f"sums_{sfx}")
            for k in range(4):
                nc.tensor.matmul(ps_sum, ones_t, a1[:, k, :], start=(k == 0),
                                 stop=(k == 3))
            # -------- Square block: mem row squares + LN squares (one ACT
            # table visit per t; Sqrt/Relu follow adjacently) --------
            sqp = []
            for bt in range(NBT):
                pr = ppool.tile([128, N, M], BF, name=f"sqp_{sfx}_{bt}",
                                tag="sqp", bufs=1)
                nc.scalar.square(pr, mem[(t, bt)])
                sqp.append(pr)
            sq4 = apool.tile([128, HC, BS], BF, name=f"sq4_{sfx}", tag="sq4",
                             bufs=1)
            for k in range(4):
                nc.scalar.square(sq4[:, k, :], a1[:, k, :])
            ps_sq = mm_ps([128, BS], f"sumsq_{sfx}")
            for k in range(4):
                nc.tensor.matmul(ps_sq, ones_b, sq4[:, k, :], start=(k == 0),
                                 stop=(k == 3))

            mu = apool.tile([128, BS], FP, name=f"mu_{sfx}", tag="mu", bufs=1)
            nc.vector.tensor_scalar(out=mu, in0=ps_sum, scalar1=1.0 / H,
                                    scalar2=None, op0=ALU.mult)
            var = apool.tile([128, BS], FP, name=f"var_{sfx}", tag="var", bufs=1)
            nc.vector.tensor_mul(var, mu, mu)
            nc.vector.scalar_tensor_tensor(out=var, in0=ps_sq, scalar=1.0 / H,
                                           in1=var, op0=ALU.mult,
                                           op1=ALU.subtract)
            nc.vector.tensor_scalar(out=var, in0=var, scalar1=1e-5,
                                    scalar2=None, op0=ALU.add)
            nc.vector.reciprocal(out=var, in_=var)
            rstd = apool.tile([128, BS], FP, name=f"rstd_{sfx}", tag="rstd",
                              bufs=1)
            nc.scalar.activation(out=rstd, in_=var, func=AF.Sqrt)

            nc.vector.tensor_sub(a1, a1, _bcast_mid(mu, HC))
            nc.vector.tensor_mul(a1, a1, _bcast_mid(rstd, HC))
            lnt = []
            for hc in range(HC):
                lt = apool.tile([128, BS], BF, name=f"lnt_{sfx}_{hc}", tag="lnt",
                                bufs=4)
                nc.scalar.activation(out=lt, in_=a1[:, hc, :], func=AF.Relu,
                                     bias=lnb[:, hc:hc + 1],
                                     scale=lng[:, hc:hc + 1])
                lnt.append(lt)

            p = []
            for vc in range(VC):
                ps = mm_ps([128, BS], f"p_{sfx}_{vc}")
                for k in range(4):
                    nc.tensor.matmul(ps, w2[k][:, vc * 128:(vc + 1) * 128], lnt[k],
                                     start=(k == 0), stop=(k == 3))
                pt = apool.tile([128, BS], BF, name=f"p_{sfx}_{vc}", tag="p", bufs=2)
                nc.scalar.activation(out=pt, in_=ps, func=AF.Tanh,
                                     bias=b2c[:, vc:vc + 1])
                p.append(pt)

            if stage < 2:
                for vc in range(VC):
                    nc.sync.dma_start(out=outT[t, vc * 128:(vc + 1) * 128, :],
                                      in_=p[vc])
                continue

            # -------- mem row norms: tree over ACT squares (fills DVE while
            # the PE/ACT gate phase runs) --------
            for bt in range(NBT):
                n2 = apool.tile([128, N], FP, name=f"n2_{sfx}_{bt}", tag="n2",
                                bufs=4)
                tree_m(n2, sqp[bt])
                sqn[(t, bt)] = n2

            # ---- deferred output projection of t-1 (keeps PE busy during
            # t-1's addressing chain; its sigma-ACTs join t's sigmoid block) --
            if pending_out is not None:
                emit_out(*pending_out)
                pending_out = None

            # ---------------- LSTM (chain starts: needs rT_prev) ----------------
            def z_chain(oc):
                osl = slice(oc * 128, (oc + 1) * 128)
                ps = mm_ps([128, BS], f"z_{sfx}_{oc}")
                nc.tensor.matmul(ps, wih[0][:, osl], p[0], start=True, stop=False)
                nc.tensor.matmul(ps, wih[1][:, osl], p[1], start=False, stop=False)
                for k in range(4):
                    nc.tensor.matmul(ps, whh[k][:, osl], h0[k], start=False,
                                     stop=False)
                nc.tensor.matmul(ps, wih[2][:, osl], rT_prev, start=False,
                                 stop=True)
                return ps

            gates = {}
            for hc in range(HC):
                for gi in (0, 1, 3):          # sigmoid gates grouped
                    oc = gi * 4 + hc
                    ps = z_chain(oc)
                    gs = apool.tile([128, BS], BF, name=f"g_{sfx}_{oc}", tag="gt",
                                    bufs=16)
                    nc.scalar.activation(out=gs, in_=ps, func=AF.Sigmoid,
                                         bias=bzc[:, oc:oc + 1])
                    gates[(gi, hc)] = gs
            for hc in range(HC):              # tanh gates grouped
                oc = 2 * 4 + hc
                ps = z_chain(oc)
                gs = apool.tile([128, BS], BF, name=f"g_{sfx}_{oc}", tag="gt",
                                bufs=16)
                nc.scalar.activation(out=gs, in_=ps, func=AF.Tanh,
                                     bias=bzc[:, oc:oc + 1])
                gates[(2, hc)] = gs

            cc = []
            for hc in range(HC):
                t2 = apool.tile([128, BS], BF, name=f"ct2_{sfx}_{hc}", tag="ct",
                                bufs=4)
                nc.vector.tensor_mul(t2, gates[(0, hc)], gates[(2, hc)])
                gf = gates[(1, hc)]
                nc.vector.tensor_mul(gf, gf, c0b[:, hc, :])  # gf = f*c0
                nc.vector.tensor_add(t2, t2, gf)            # t2 = c
                cc.append(t2)
            tc_ = []
            for hc in range(HC):
                th = apool.tile([128, BS], BF, name=f"tc_{sfx}_{hc}", tag="tch",
                                bufs=4)
                nc.scalar.activation(out=th, in_=cc[hc], func=AF.Tanh)
                tc_.append(th)
            h = []
            for hc in range(HC):
                ht = apool.tile([128, BS], BF, name=f"h_{sfx}_{hc}", tag="h",
                                bufs=4)
                nc.vector.tensor_mul(ht, gates[(3, hc)], tc_[hc])
                h.append(ht)

            if stage < 3:
                for k in range(4):
                    nc.sync.dma_start(out=outT[t, k * 128:(k + 1) * 128, :],
                                      in_=h[k])
                continue

            # ---------------- read head: or^T = h^T Wr^T + br ----------------
            # PE with h (stationary) x wr (moving) gives [batch, M+6] directly.
            kT = []
            psor = []
            for bt in range(NBT):
                bsl = slice(bt * 128, (bt + 1) * 128)
                ps = mm_ps([128, M + 6], f"orT_{sfx}_{bt}", tag="orT", bufs=2)
                for k in range(4):
                    nc.tensor.matmul(ps, h[k][:, bsl], wr_[k], start=(k == 0),
                                     stop=False)
                nc.tensor.matmul(ps, ones_row, brr, start=False, stop=True)
                kt = apool.tile([128, M], BF, name=f"kT_{sfx}_{bt}", tag="kT",
                                bufs=2)
                nc.scalar.activation(out=kt, in_=ps[:, 0:M], func=AF.Tanh)
                kT.append(kt)
                psor.append(ps)

            rT_next = spool.tile([M, BS], BF, name=f"rT_{sfx}", tag="rT", bufs=2)

            # ---- addressing, bt-batched per ACT-table stage ----
            def sc(nm, bt, w=1):
                return apool.tile([128, w], FP, name=f"{nm}_{sfx}_{bt}",
                                  tag="sc1", bufs=24)

            # Exp of raw head scalars [beta, g, s0, s1, s2, gamma]
            khe = [sc("khe", bt, 6) for bt in range(NBT)]
            for bt in range(NBT):
                nc.scalar.activation(out=khe[bt], in_=psor[bt][:, M:M + 6],
                                     func=AF.Exp)
            uu, sp2, s3, kn2 = [], [], [], []
            for bt in range(NBT):
                u = sc("u", bt)
                nc.vector.tensor_scalar(out=u, in0=khe[bt][:, 1:2], scalar1=1.0,
                                        scalar2=None, op0=ALU.add)
                nc.vector.reciprocal(out=u, in_=u)          # u = 1-sig(g)
                uu.append(u)
                v2 = sc("sp2", bt, 2)
                nc.vector.tensor_scalar(out=v2, in0=_cols(khe[bt], 0, 5, 2),
                                        scalar1=1.0, scalar2=None, op0=ALU.add)
                sp2.append(v2)                              # [1+e^b, 1+e^gam]
                ssum = sc("ssum", bt)
                nc.vector.reduce_sum(out=ssum, in_=khe[bt][:, 2:5], axis=AX.X)
                nc.vector.reciprocal(out=ssum, in_=ssum)
                s3t = sc("s3", bt, 3)
                nc.vector.tensor_scalar(out=s3t, in0=khe[bt][:, 2:5],
                                        scalar1=ssum, scalar2=None, op0=ALU.mult)
                s3.append(s3t)
                kk = sc("kn2", bt)
                ksc = ppool.tile([128, M], FP, name=f"ksc_{sfx}_{bt}", tag="ksc",
                                 bufs=1)
                nc.vector.tensor_mul(ksc, kT[bt], kT[bt])
                nc.vector.reduce_sum(out=kk, in_=ksc, axis=AX.X)
                kn2.append(kk)

            # Ln block: softplus(beta/gamma) then ln(beta); ln(n2*k2)
            lsp, lnbeta, lden = [], [], []
            for bt in range(NBT):
                ls = sc("lsp", bt, 2)
                nc.scalar.activation(out=ls, in_=sp2[bt], func=AF.Ln)
                lsp.append(ls)                               # [beta, sp(gamma)]
            for bt in range(NBT):
                lb = sc("lnbeta", bt)
                nc.scalar.activation(out=lb, in_=lsp[bt][:, 0:1], func=AF.Ln)
                lnbeta.append(lb)
            for bt in range(NBT):
                n2k2 = apool.tile([128, N], FP, name=f"n2k2_{sfx}_{bt}",
                                  tag="n2k2", bufs=2)
                nc.vector.tensor_scalar(out=n2k2, in0=sqn[(t, bt)],
                                        scalar1=kn2[bt],
                                        scalar2=None, op0=ALU.mult)
                ld = apool.tile([128, N], FP, name=f"lden_{sfx}_{bt}",
                                tag="lden", bufs=2)
                nc.scalar.activation(out=ld, in_=n2k2, func=AF.Ln, bias=eps30)
                lden.append(ld)

            gam = []
            for bt in range(NBT):
                g = sc("gam", bt)
                nc.vector.tensor_scalar(out=g, in0=lsp[bt][:, 1:2], scalar1=1.0,
                                        scalar2=None, op0=ALU.add)
                gam.append(g)

            # cos numerator: mid-dim k broadcast runs in DVE 2x mode
            cn = []
            for bt in range(NBT):
                prod = ppool.tile([128, N, M], BF, name=f"prodc_{sfx}_{bt}",
                                  tag="prod", bufs=1)
                nc.vector.tensor_mul(prod, mem[(t, bt)],
                                     _bcast_mid(kT[bt], N))
                cnt = apool.tile([128, N], FP, name=f"cn_{sfx}_{bt}", tag="cn",
                                 bufs=2)
                tree_m(cnt, prod)
                cn.append(cnt)

            # Exp block: lw = exp(-0.5*ln(n2k2) + ln(beta)); e = exp(lw*cn)
            for bt in range(NBT):
                lw = apool.tile([128, N], FP, name=f"lw_{sfx}_{bt}", tag="lw",
                                bufs=2)
                nc.scalar.activation(out=lw, in_=lden[bt], func=AF.Exp,
                                     scale=-0.5, bias=lnbeta[bt])
                nc.vector.tensor_mul(cn[bt], cn[bt], lw)     # beta*cos logits
            ee = []
            for bt in range(NBT):
                e = apool.tile([128, N], FP, name=f"e_{sfx}_{bt}", tag="e",
                               bufs=2)
                nc.scalar.activation(out=e, in_=cn[bt], func=AF.Exp)
                ee.append(e)

            if stage < 45:
                for bt in range(NBT):
                    nc.sync.dma_start(
                        out=outT[t, bt * 128:(bt + 1) * 128, 0:N], in_=ee[bt])
                continue

            # wg = (g/sum_e)*e + (1-g)*w_prev, then 3-tap circular shift
            ws = []
            for bt in range(NBT):
                se = sc("se", bt)
                nc.vector.reduce_sum(out=se, in_=ee[bt], axis=AX.X)
                nc.vector.reciprocal(out=se, in_=se)
                gsig = sc("gsig", bt)
                nc.vector.tensor_scalar(out=gsig, in0=uu[bt], scalar1=-1.0,
                                        scalar2=1.0, op0=ALU.mult, op1=ALU.add)
                nc.vector.tensor_mul(gsig, gsig, se)         # g / sum_e
                w0p = apool.tile([128, N], FP, name=f"w0p_{sfx}_{bt}",
                                 tag="w0p", bufs=2)
                nc.vector.tensor_scalar(out=w0p, in0=w0[bt], scalar1=uu[bt],
                                        scalar2=None, op0=ALU.mult)
                wg = apool.tile([128, N], FP, name=f"wg_{sfx}_{bt}", tag="wg",
                                bufs=2)
                nc.vector.scalar_tensor_tensor(out=wg, in0=ee[bt],
                                               scalar=gsig, in1=w0p,
                                               op0=ALU.mult, op1=ALU.add)
                # ws = s0*roll(wg,+1) + s1*wg + s2*roll(wg,-1)
                wmid = apool.tile([128, N], FP, name=f"wmid_{sfx}_{bt}",
                                  tag="wmid", bufs=2)
                nc.vector.tensor_scalar(out=wmid, in0=wg, scalar1=s3[bt][:, 1:2],
                                        scalar2=None, op0=ALU.mult)
                wst = apool.tile([128, N], FP, name=f"ws_{sfx}_{bt}", tag="ws",
                                 bufs=2)
                nc.vector.scalar_tensor_tensor(out=wst[:, 1:N], in0=wg[:, 0:N - 1],
                                               scalar=s3[bt][:, 0:1],
                                               in1=wmid[:, 1:N],
                                               op0=ALU.mult, op1=ALU.add)
                nc.vector.scalar_tensor_tensor(out=wst[:, 0:1], in0=wg[:, N - 1:N],
                                               scalar=s3[bt][:, 0:1],
                                               in1=wmid[:, 0:1],
                                               op0=ALU.mult, op1=ALU.add)
                nc.vector.scalar_tensor_tensor(out=wmid[:, 0:N - 1],
                                               in0=wg[:, 1:N],
                                               scalar=s3[bt][:, 2:3],
                                               in1=wst[:, 0:N - 1],
                                               op0=ALU.mult, op1=ALU.add)
                nc.vector.scalar_tensor_tensor(out=wmid[:, N - 1:N],
                                               in0=wg[:, 0:1],
                                               scalar=s3[bt][:, 2:3],
                                               in1=wst[:, N - 1:N],
                                               op0=ALU.mult, op1=ALU.add)
                ws.append(wmid)

            # sharpen: wp = ws**gamma (unnormalised; fold 1/sum into r)
            lnws = []
            for bt in range(NBT):
                lt = apool.tile([128, N], FP, name=f"lnws_{sfx}_{bt}",
                                tag="lnws", bufs=2)
                nc.scalar.activation(out=lt, in_=ws[bt], func=AF.Ln, bias=eps30)
                nc.vector.tensor_scalar(out=lt, in0=lt, scalar1=gam[bt],
                                        scalar2=None, op0=ALU.mult)
                lnws.append(lt)
            wp = []
            for bt in range(NBT):
                wpt = apool.tile([128, N], BF, name=f"wp_{sfx}_{bt}", tag="wp",
                                 bufs=2)
                nc.scalar.activation(out=wpt, in_=lnws[bt], func=AF.Exp)
                wp.append(wpt)

            # r = sum_n wp*mem / sum_n wp
            for bt in range(NBT):
                bsl = slice(bt * 128, (bt + 1) * 128)
                swp = sc("swp", bt)
                nc.vector.reduce_sum(out=swp, in_=wp[bt], axis=AX.X)
                nc.vector.reciprocal(out=swp, in_=swp)
                prodr = ppool.tile([128, N, M], BF, name=f"prodr_{sfx}_{bt}",
                                   tag="prod", bufs=1)
                for nh in range(2):
                    nsl = slice(nh * (N // 2), (nh + 1) * (N // 2))
                    nc.vector.tensor_mul(prodr[:, nsl, :],
                                         mem[(t, bt)][:, nsl, :],
                                         _bcast_inner(wp[bt][:, nsl], M))
                rp = apool.tile([128, 1, M], FP, name=f"rp_{sfx}_{bt}",
                                tag="rp", bufs=2)
                tree_n(rp, prodr)
                rfin = apool.tile([128, M], FP, name=f"rfin_{sfx}_{bt}",
                                  tag="rfin", bufs=2)
                nc.vector.tensor_scalar(out=rfin, in0=rp[:, 0, :], scalar1=swp,
                                        scalar2=None, op0=ALU.mult)
                pst = mm_ps([M, 128], f"rtp_{sfx}_{bt}", tag="tp", bufs=1)
                nc.tensor.transpose(pst, rfin, ident)
                nc.vector.tensor_copy(out=rT_next[:, bsl], in_=pst)

            if stage < 99:
                nc.sync.dma_start(out=outT[t, 0:M, :], in_=rT_next)
                rT_prev = rT_next
                continue

            # output projection of this t is deferred into iteration t+1
            pending_out = (t, wo, h, rT_next, boc)
            rT_prev = rT_next

        if pending_out is not None and stage >= 99:
            emit_out(*pending_out)

    nc.compile()
    return nc


_CACHE = {}
LAST = {}


def _get_nc():
    if "nc" not in _CACHE:
        _CACHE["nc"] = build_nc()
    return _CACHE["nc"]


def host_prep(inputs, W1, b1, lng, lnb, W2, b2, Wih, Whh, bih, bhh,
              Wr, br, Ww, bw, Wo, bo, mem0, read0, wr0, ww0, h0, c0):
    f32 = np.float32
    inputs, W1, W2, Wih, Whh, Wr, Wo = [np.asarray(a, f32) for a in
                                        (inputs, W1, W2, Wih, Whh, Wr, Wo)]

    def percol(v, cols):   # [T, 128*cols] -> [T, 128, cols] column-major chunks
        return np.ascontiguousarray(
            np.asarray(v, f32).reshape(T, cols, 128).transpose(0, 2, 1))

    bf = ml_dtypes.bfloat16
    xT_full = np.ascontiguousarray(inputs.transpose(0, 2, 1)).astype(bf)
    w1t = np.ascontiguousarray(W1.transpose(0, 2, 1)).astype(bf)   # [T, E, H]
    w2t = np.ascontiguousarray(W2.transpose(0, 2, 1)).astype(bf)   # [T, H, V]
    wiht = np.ascontiguousarray(Wih.transpose(0, 2, 1)).astype(bf)
    whht = np.ascontiguousarray(Whh.transpose(0, 2, 1)).astype(bf)
    wrt = np.ascontiguousarray(Wr.transpose(0, 2, 1)).astype(bf)   # [T, H, 70]
    wot = np.ascontiguousarray(Wo.transpose(0, 2, 1)).astype(bf)   # [T, 576, E]
    h0t_full = np.asarray(h0, f32).transpose(0, 2, 1).astype(bf)
    c0t_full = np.asarray(c0, f32).transpose(0, 2, 1).astype(bf)
    r0t_full = np.asarray(read0, f32)[T - 1].T.astype(bf)          # [M, B]
    wr0_full = np.asarray(wr0, f32).astype(bf)
    mem0_full = np.asarray(mem0).astype(bf)
    bz = np.asarray(bih, f32) + np.asarray(bhh, f32)

    common = dict(
        w1t=w1t, w2t=w2t, wiht=wiht, whht=whht, wrt=wrt, wot=wot,
        b1c=percol(b1, HC), lngc=percol(lng, HC), lnbc=percol(lnb, HC),
        b2c=percol(b2, VC), bzc=percol(bz, ZC),
        brr=np.ascontiguousarray(np.asarray(br, f32).reshape(T, 1, M + 6)),
        boc=percol(bo, EC),
    )
    in_maps = []
    for ci in range(NCORES):
        bsl = slice(ci * BS, (ci + 1) * BS)
        in_maps.append(dict(
            common,
            xT=np.ascontiguousarray(xT_full[:, :, bsl]),
            h0t=np.ascontiguousarray(h0t_full[:, :, bsl]),
            c0t=np.ascontiguousarray(c0t_full[:, :, bsl]),
            r0t=np.ascontiguousarray(r0t_full[:, bsl]),
            wr0=np.ascontiguousarray(wr0_full[:, bsl, :]),
            mem0=np.ascontiguousarray(mem0_full[:, bsl]),
        ))

    return in_maps


def kernel(**inputs):
    in_maps = host_prep(**inputs)
    nc = _get_nc()
    import os
    trace = os.environ.get("BASS_TRACE", "") not in ("", "0")
    res = run_bass_kernel_spmd(nc, in_maps, list(range(NCORES)), trace=trace)
    LAST["exec_time_ns"] = res.exec_time_ns
    LAST["results"] = res
    out = np.concatenate(
        [np.transpose(r["outT"], (0, 2, 1)) for r in res.results], axis=1)
    return np.ascontiguousarray(out.astype(np.float32))


# revision 39
# speedup vs baseline: 1.0058x; 1.0058x over previous
"""Trainium2 Bass kernel for nn_CM_NTM_29566554866014 (scatter_memory).

Sharding: pure batch data-parallelism across 8 NeuronCores (B=2048 -> 256/core).
Small parameters replicated. The cross-NTM loop (T=4) is sequential but
batch-local, so each core runs all 4 steps on its batch shard independently.
No collectives.

Key structural facts used (verified against the reference math):
  * The write head (Ww/bw/ww0) and the memory erase/add update are dead code:
    `mem` is reassigned to `mem0[i+1]` each iteration and outputs depend only
    on h and r. They are therefore not computed.
  * Only read0[T-1] is consumed.
  * Per-step state (mem0/h0/c0/wr0) are fresh inputs each step; the only
    sequential dependency across steps is the read vector r.

Engine assignment (v5, evolved via perfetto traces; 524us -> ~426us):
  * DVE (vector) is the bottleneck engine; every big elementwise op is bf16
    so it runs in DVE 2x mode. Empirical 2x rule on TRN2: a stride-0
    broadcast on a MIDDLE free dim keeps 2x (cos numerator: k broadcast
    over n), a stride-0 broadcast on the INNERMOST dim drops to 1x (read
    weighting: w broadcast over m - unavoidable in this layout).
  * gpsimd is a trap here: it shares SBUF read/write ports with the DVE,
    so offloading big elementwise ops to it inflates concurrent DVE ops
    3-5x. All elementwise stays on DVE/ACT.
  * mem row-norm squares run on the ACT engine (Square), grouped with the
    LN squares into one table visit; activation calls are grouped per
    function to minimise ACT table reloads (1.28us each).
  * addressing math is Exp/Ln-only: 1/(|m||k|) and *beta folded into one
    Exp(-0.5*ln(n2*k2) + ln(beta)) ACT op; softmax max-subtraction dropped
    (logits bounded); softplus via shared Exp/Ln blocks; sharpen left
    unnormalised and the 1/sum folded into the read vector.
  * read-head output computed transposed on the PE (h as stationary), which
    removes the k/kh PE transposes + drains; bias via a ones-row matmul.
  * output projection of step t is deferred into step t+1 so the PE runs
    proj(t+1) while the DVE walks t's addressing chain.
  * tensor_tensor_reduce crashes the HW runtime (NRT_EXEC_UNIT_UNRECOVERABLE)
    - do not use it.
"""

import numpy as np
import ml_dtypes
from contextlib import ExitStack

import concourse.bass as bass
import concourse.tile as tile
from concourse import bacc
from concourse import mybir
from concourse.bass_utils import run_bass_kernel_spmd
from concourse.masks import make_identity

AF = mybir.ActivationFunctionType
ALU = mybir.AluOpType
AX = mybir.AxisListType
FP = mybir.dt.float32
BF = mybir.dt.bfloat16

T, E, V, H, N, M, B = 4, 512, 256, 512, 128, 64, 2048
NCORES = 8
BS = B // NCORES      # 256 batch rows per core
NBT = BS // 128       # 2 batch tiles
HC = H // 128         # 4
EC = E // 128         # 4
VC = V // 128         # 2
ZC = (4 * H) // 128   # 16


def _bcast_inner(ap, count):
    """View `ap` ([P, F]) as [P, F, count] with a stride-0 innermost dim."""
    return bass.AP(tensor=ap.tensor, offset=ap.offset,
                   ap=[*ap.ap, [0, count]])


def _bcast_mid(ap, count):
    """View `ap` ([P, F]) as [P, count, F] with a stride-0 middle dim."""
    return bass.AP(tensor=ap.tensor, offset=ap.offset,
                   ap=[ap.ap[0], [0, count], ap.ap[1]])


def _cols(ap, start, step, count):
    """Strided column view of a 2-dim AP: columns start, start+step, ..."""
    st = ap.ap[1][0]
    return bass.AP(tensor=ap.tensor, offset=ap.offset + start * st,
                   ap=[ap.ap[0], [step * st, count]])


def build_nc(stage=None):
    import os
    if stage is None:
        stage = int(os.environ.get("NTM_STAGE", "99"))
    nc = bacc.Bacc()
    d = {}

    def din(name, shape, dt=FP):
        d[name] = nc.dram_tensor(name, list(shape), dt, kind="ExternalInput")

    din("xT",   (T, E, BS), BF)
    din("w1t",  (T, E, H), BF)
    din("w2t",  (T, H, V), BF)
    din("wiht", (T, V + M, 4 * H), BF)
    din("whht", (T, H, 4 * H), BF)
    din("wrt",  (T, H, M + 6), BF)
    din("wot",  (T, H + M, E), BF)
    din("h0t",  (T, H, BS), BF)
    din("c0t",  (T, H, BS), BF)
    din("r0t",  (M, BS), BF)
    din("wr0",  (T, BS, N), BF)
    din("mem0", (T, BS, N, M), BF)
    din("b1c",  (T, 128, HC))
    din("lngc", (T, 128, HC))
    din("lnbc", (T, 128, HC))
    din("b2c",  (T, 128, VC))
    din("bzc",  (T, 128, ZC))
    din("brr",  (T, 1, M + 6))
    din("boc",  (T, 128, EC))
    outT = nc.dram_tensor("outT", [T, E, BS], FP, kind="ExternalOutput")

    with tile.TileContext(nc) as tc, ExitStack() as ctx:
        singles = ctx.enter_context(tc.tile_pool(name="singles", bufs=1))
        wpool = ctx.enter_context(tc.tile_pool(name="wpool", bufs=1))
        spool = ctx.enter_context(tc.tile_pool(name="spool", bufs=1))
        apool = ctx.enter_context(tc.tile_pool(name="apool", bufs=1))
        mpool = ctx.enter_context(tc.tile_pool(name="mpool", bufs=1))
        ppool = ctx.enter_context(tc.tile_pool(name="ppool", bufs=1))
        pmm = ctx.enter_context(tc.tile_pool(name="pmm", bufs=1, space="PSUM"))

        ones_t = singles.tile([128, 128], FP, name="ones_t")
        nc.vector.memset(ones_t, 1.0)
        ones_row = singles.tile([1, 128], FP, name="ones_row")
        nc.vector.memset(ones_row, 1.0)
        ones_b = singles.tile([128, 128], BF, name="ones_b")
        nc.vector.memset(ones_b, 1.0)
        ident = singles.tile([128, 128], FP, name="ident")
        make_identity(nc, ident)
        eps30 = singles.tile([128, 1], FP, name="eps30")
        nc.vector.memset(eps30, 1e-30)

        def mm_ps(shape, name, tag="mm", bufs=5):
            return pmm.tile(shape, FP, name=name, tag=tag, bufs=bufs)

        def tree_m(dst2d, prod, eng=None):
            """Sum prod [128, N, M] over innermost m into dst2d [128, N] fp32
            via pairwise bf16 adds (DVE 2x mode)."""
            eng = eng or nc.vector
            G = prod.shape[1]
            s1 = ppool.tile([128, G, M // 2], BF, name="trm", tag="trm", bufs=1)
            eng.tensor_add(s1, prod[:, :, 0:M // 2], prod[:, :, M // 2:M])
            w = M // 2
            while w > 2:
                hw = w // 2
                eng.tensor_add(s1[:, :, 0:hw], s1[:, :, 0:hw], s1[:, :, hw:w])
                w = hw
            dst3 = bass.AP(tensor=dst2d.tensor, offset=dst2d.offset,
                           ap=[*dst2d.ap, [1, 1]])
            eng.tensor_add(dst3, s1[:, :, 0:1], s1[:, :, 1:2])

        def tree_n(dst3d, prod):
            """Sum prod [128, N, M] over mid n into dst3d [128, 1, M] fp32
            via pairwise bf16 adds on contiguous halves."""
            G = prod.shape[1]
            s1 = ppool.tile([128, G // 2, M], BF, name="trn", tag="trn", bufs=1)
            nc.vector.tensor_add(s1, prod[:, 0:G // 2, :], prod[:, G // 2:G, :])
            w = G // 2
            while w > 2:
                hw = w // 2
                nc.vector.tensor_add(s1[:, 0:hw, :], s1[:, 0:hw, :],
                                     s1[:, hw:w, :])
                w = hw
            nc.vector.tensor_add(dst3d, s1[:, 0:1, :], s1[:, 1:2, :])

        mem = {}
        sqn = {}

        def load_mem(t):
            for bt in range(NBT):
                mt = mpool.tile([128, N, M], BF, name=f"mem_t{t}_{bt}",
                                tag="mem", bufs=3)
                nc.sync.dma_start(out=mt, in_=d["mem0"][t, bt * 128:(bt + 1) * 128])
                mem[(t, bt)] = mt

        def emit_out(to, wo_, h_, rT_, boc_):
            for ec in range(EC):
                esl = slice(ec * 128, (ec + 1) * 128)
                ps = mm_ps([128, BS], f"o_t{to}_{ec}")
                for k in range(4):
                    nc.tensor.matmul(ps, wo_[k][:, esl], h_[k], start=(k == 0),
                                     stop=False)
                nc.tensor.matmul(ps, wo_[4][:, esl], rT_, start=False,
                                 stop=True)
                os_ = apool.tile([128, BS], FP, name=f"os_t{to}_{ec}", tag="os",
                                 bufs=2)
                nc.scalar.activation(out=os_, in_=ps, func=AF.Sigmoid,
                                     bias=boc_[:, ec:ec + 1])
                nc.sync.dma_start(out=outT[to, esl, :], in_=os_)

        pending_out = None
        rT_prev = None
        for t in range(T):
            sfx = f"t{t}"
            # ---------------- loads ----------------
            # Issue order matters: the sync engine fires DMAs in program
            # order, so load what this t needs first (proj inputs, mem for
            # the ACT squares), then the late-phase weights, then prefetch.
            w1 = [wpool.tile([128, H], BF, name=f"w1_{sfx}_{k}", tag="w1",
                             bufs=4) for k in range(4)]
            for k in range(4):
                nc.sync.dma_start(out=w1[k], in_=d["w1t"][t, k * 128:(k + 1) * 128, :])
            xT = [spool.tile([128, BS], BF, name=f"xT_{sfx}_{k}", tag="xT",
                             bufs=4) for k in range(4)]
            for k in range(4):
                nc.sync.dma_start(out=xT[k], in_=d["xT"][t, k * 128:(k + 1) * 128, :])
            b1c = spool.tile([128, HC], FP, name=f"b1c_{sfx}", tag="b1c", bufs=2)
            lng = spool.tile([128, HC], FP, name=f"lng_{sfx}", tag="lng", bufs=2)
            lnb = spool.tile([128, HC], FP, name=f"lnb_{sfx}", tag="lnb", bufs=2)
            b2c = spool.tile([128, VC], FP, name=f"b2c_{sfx}", tag="b2c", bufs=2)
            bzc = spool.tile([128, ZC], FP, name=f"bzc_{sfx}", tag="bzc", bufs=2)
            brr = spool.tile([1, M + 6], FP, name=f"brr_{sfx}", tag="brr", bufs=2)
            boc = spool.tile([128, EC], FP, name=f"boc_{sfx}", tag="boc", bufs=2)
            nc.sync.dma_start(out=b1c, in_=d["b1c"][t])
            nc.sync.dma_start(out=lng, in_=d["lngc"][t])
            nc.sync.dma_start(out=lnb, in_=d["lnbc"][t])
            nc.sync.dma_start(out=b2c, in_=d["b2c"][t])
            if t == 0:
                load_mem(0)
            w2 = [wpool.tile([128, V], BF, name=f"w2_{sfx}_{k}", tag="w2",
                             bufs=4) for k in range(4)]
            for k in range(4):
                nc.sync.dma_start(out=w2[k], in_=d["w2t"][t, k * 128:(k + 1) * 128, :])
            h0 = [spool.tile([128, BS], BF, name=f"h0_{sfx}_{k}", tag="h0",
                             bufs=4) for k in range(4)]
            c0b = spool.tile([128, HC, BS], BF, name=f"c0b_{sfx}", tag="c0",
                             bufs=2)
            for k in range(4):
                nc.sync.dma_start(out=h0[k], in_=d["h0t"][t, k * 128:(k + 1) * 128, :])
                nc.sync.dma_start(out=c0b[:, k, :],
                                  in_=d["c0t"][t, k * 128:(k + 1) * 128, :])
            nc.sync.dma_start(out=bzc, in_=d["bzc"][t])
            wih = []
            for k, ksz in enumerate((128, 128, 64)):
                wt = wpool.tile([ksz, 4 * H], BF, name=f"wih_{sfx}_{k}", tag="wih",
                                bufs=3)
                nc.sync.dma_start(out=wt, in_=d["wiht"][t, k * 128:k * 128 + ksz, :])
                wih.append(wt)
            whh = [wpool.tile([128, 4 * H], BF, name=f"whh_{sfx}_{k}", tag="whh",
                              bufs=4) for k in range(4)]
            for k in range(4):
                nc.sync.dma_start(out=whh[k], in_=d["whht"][t, k * 128:(k + 1) * 128, :])
            wr_ = [wpool.tile([128, M + 6], BF, name=f"wr_{sfx}_{k}", tag="wr",
                              bufs=4) for k in range(4)]
            for k in range(4):
                nc.sync.dma_start(out=wr_[k], in_=d["wrt"][t, k * 128:(k + 1) * 128, :])
            nc.sync.dma_start(out=brr, in_=d["brr"][t])
            w0 = []
            for bt in range(NBT):
                wt = spool.tile([128, N], BF, name=f"w0_{sfx}_{bt}", tag="w0",
                                bufs=4)
                nc.sync.dma_start(out=wt, in_=d["wr0"][t, bt * 128:(bt + 1) * 128, :])
                w0.append(wt)
            wo = []
            for k, ksz in enumerate((128, 128, 128, 128, 64)):
                wt = wpool.tile([ksz, E], BF, name=f"wo_{sfx}_{k}", tag="wo", bufs=5)
                nc.sync.dma_start(out=wt, in_=d["wot"][t, k * 128:k * 128 + ksz, :])
                wo.append(wt)
            nc.sync.dma_start(out=boc, in_=d["boc"][t])
            if t + 1 < T:
                load_mem(t + 1)

            if t == 0:
                rT_prev = spool.tile([M, BS], BF, name="r0T", tag="rT", bufs=2)
                nc.sync.dma_start(out=rT_prev, in_=d["r0t"][:, :])

            # ---------------- input projection + LN + p ----------------
            a1 = apool.tile([128, HC, BS], FP, name=f"a1_{sfx}", tag="a1", bufs=1)
            for hc in range(HC):
                ps = mm_ps([128, BS], f"a1_{sfx}_{hc}")
                for k in range(4):
                    nc.tensor.matmul(ps, w1[k][:, hc * 128:(hc + 1) * 128], xT[k],
                                     start=(k == 0), stop=(k == 3))
                nc.vector.tensor_scalar(out=a1[:, hc, :], in0=ps,
                                        scalar1=b1c[:, hc:hc + 1], scalar2=None,
                                        op0=ALU.add)

            ps_sum = mm_ps([128, BS], f"sums_{sfx}")
            for k in range(4):
                nc.tensor.matmul(ps_sum, ones_t, a1[:, k, :], start=(k == 0),
                                 stop=(k == 3))
            # -------- Square block: mem row squares + LN squares (one ACT
            # table visit per t; Sqrt/Relu follow adjacently) --------
            sqp = []
            for bt in range(NBT):
                pr = ppool.tile([128, N, M], BF, name=f"sqp_{sfx}_{bt}",
                                tag="sqp", bufs=2)
                nc.scalar.square(pr, mem[(t, bt)])
                sqp.append(pr)
            sq4 = apool.tile([128, HC, BS], BF, name=f"sq4_{sfx}", tag="sq4",
                             bufs=1)
            for k in range(4):
                nc.scalar.square(sq4[:, k, :], a1[:, k, :])
            ps_sq = mm_ps([128, BS], f"sumsq_{sfx}")
            for k in range(4):
                nc.tensor.matmul(ps_sq, ones_b, sq4[:, k, :], start=(k == 0),
                                 stop=(k == 3))

            mu = apool.tile([128, BS], FP, name=f"mu_{sfx}", tag="mu", bufs=1)
            nc.vector.tensor_scalar(out=mu, in0=ps_sum, scalar1=1.0 / H,
                                    scalar2=None, op0=ALU.mult)
            var = apool.tile([128, BS], FP, name=f"var_{sfx}", tag="var", bufs=1)
            nc.vector.tensor_mul(var, mu, mu)
            nc.vector.scalar_tensor_tensor(out=var, in0=ps_sq, scalar=1.0 / H,
                                           in1=var, op0=ALU.mult,
                                           op1=ALU.subtract)
            nc.vector.tensor_scalar(out=var, in0=var, scalar1=1e-5,
                                    scalar2=None, op0=ALU.add)
            nc.vector.reciprocal(out=var, in_=var)
            rstd = apool.tile([128, BS], FP, name=f"rstd_{sfx}", tag="rstd",
                              bufs=1)
            nc.scalar.activation(out=rstd, in_=var, func=AF.Sqrt)

            nc.vector.tensor_sub(a1, a1, _bcast_mid(mu, HC))
            nc.vector.tensor_mul(a1, a1, _bcast_mid(rstd, HC))
            lnt = []
            for hc in range(HC):
                lt = apool.tile([128, BS], BF, name=f"lnt_{sfx}_{hc}", tag="lnt",
                                bufs=4)
                nc.scalar.activation(out=lt, in_=a1[:, hc, :], func=AF.Relu,
                                     bias=lnb[:, hc:hc + 1],
                                     scale=lng[:, hc:hc + 1])
                lnt.append(lt)

            p = []
            for vc in range(VC):
                ps = mm_ps([128, BS], f"p_{sfx}_{vc}")
                for k in range(4):
                    nc.tensor.matmul(ps, w2[k][:, vc * 128:(vc + 1) * 128], lnt[k],
                                     start=(k == 0), stop=(k == 3))
                pt = apool.tile([128, BS], BF, name=f"p_{sfx}_{vc}", tag="p", bufs=2)
                nc.scalar.activation(out=pt, in_=ps, func=AF.Tanh,
                                     bias=b2c[:, vc:vc + 1])
                p.append(pt)

            if stage < 2:
                for vc in range(VC):
                    nc.sync.dma_start(out=outT[t, vc * 128:(vc + 1) * 128, :],
                                      in_=p[vc])
                continue

            # -------- mem row norms: tree over ACT squares (fills DVE while
            # the PE/ACT gate phase runs) --------
            for bt in range(NBT):
                n2 = apool.tile([128, N], FP, name=f"n2_{sfx}_{bt}", tag="n2",
                                bufs=4)
                tree_m(n2, sqp[bt])
                sqn[(t, bt)] = n2

            # ---- deferred output projection of t-1 (keeps PE busy during
            # t-1's addressing chain; its sigma-ACTs join t's sigmoid block) --
            if pending_out is not None:
                emit_out(*pending_out)
                pending_out = None

            # ---------------- LSTM (chain starts: needs rT_prev) ----------------
            def z_chain(oc):
                osl = slice(oc * 128, (oc + 1) * 128)
                ps = mm_ps([128, BS], f"z_{sfx}_{oc}")
                nc.tensor.matmul(ps, wih[0][:, osl], p[0], start=True, stop=False)
                nc.tensor.matmul(ps, wih[1][:, osl], p[1], start=False, stop=False)
                for k in range(4):
                    nc.tensor.matmul(ps, whh[k][:, osl], h0[k], start=False,
                                     stop=False)
                nc.tensor.matmul(ps, wih[2][:, osl], rT_prev, start=False,
                                 stop=True)
                return ps

            gates = {}
            for hc in range(HC):
                for gi in (0, 1, 3):          # sigmoid gates grouped
                    oc = gi * 4 + hc
                    ps = z_chain(oc)
                    gs = apool.tile([128, BS], BF, name=f"g_{sfx}_{oc}", tag="gt",
                                    bufs=16)
                    nc.scalar.activation(out=gs, in_=ps, func=AF.Sigmoid,
                                         bias=bzc[:, oc:oc + 1])
                    gates[(gi, hc)] = gs
            for hc in range(HC):              # tanh gates grouped
                oc = 2 * 4 + hc
                ps = z_chain(oc)
                gs = apool.tile([128, BS], BF, name=f"g_{sfx}_{oc}", tag="gt",
                                bufs=16)
                nc.scalar.activation(out=gs, in_=ps, func=AF.Tanh,
                                     bias=bzc[:, oc:oc + 1])
                gates[(2, hc)] = gs

            cc = []
            for hc in range(HC):
                t2 = apool.tile([128, BS], BF, name=f"ct2_{sfx}_{hc}", tag="ct",
                                bufs=4)
                nc.vector.tensor_mul(t2, gates[(0, hc)], gates[(2, hc)])
                gf = gates[(1, hc)]
                nc.vector.tensor_mul(gf, gf, c0b[:, hc, :])  # gf = f*c0
                nc.vector.tensor_add(t2, t2, gf)            # t2 = c
                cc.append(t2)
            tc_ = []
            for hc in range(HC):
                th = apool.tile([128, BS], BF, name=f"tc_{sfx}_{hc}", tag="tch",
                                bufs=4)
                nc.scalar.activation(out=th, in_=cc[hc], func=AF.Tanh)
                tc_.append(th)
            h = []
            for hc in range(HC):
                ht = apool.tile([128, BS], BF, name=f"h_{sfx}_{hc}", tag="h",
                                bufs=4)
                nc.vector.tensor_mul(ht, gates[(3, hc)], tc_[hc])
                h.append(ht)

            if stage < 3:
                for k in range(4):
                    nc.sync.dma_start(out=outT[t, k * 128:(k + 1) * 128, :],
                                      in_=h[k])
                continue

            # ---------------- read head: or^T = h^T Wr^T + br ----------------
            # PE with h (stationary) x wr (moving) gives [batch, M+6] directly.
            kT = []
            psor = []
            for bt in range(NBT):
                bsl = slice(bt * 128, (bt + 1) * 128)
                ps = mm_ps([128, M + 6], f"orT_{sfx}_{bt}", tag="orT", bufs=2)
                for k in range(4):
                    nc.tensor.matmul(ps, h[k][:, bsl], wr_[k], start=(k == 0),
                                     stop=False)
                nc.tensor.matmul(ps, ones_row, brr, start=False, stop=True)
                kt = apool.tile([128, M], BF, name=f"kT_{sfx}_{bt}", tag="kT",
                                bufs=2)
                nc.scalar.activation(out=kt, in_=ps[:, 0:M], func=AF.Tanh)
                kT.append(kt)
                psor.append(ps)

            rT_next = spool.tile([M, BS], BF, name=f"rT_{sfx}", tag="rT", bufs=2)

            # ---- addressing, bt-batched per ACT-table stage ----
            def sc(nm, bt, w=1):
                return apool.tile([128, w], FP, name=f"{nm}_{sfx}_{bt}",
                                  tag="sc1", bufs=24)

            # Exp of raw head scalars [beta, g, s0, s1, s2, gamma]
            khe = [sc("khe", bt, 6) for bt in range(NBT)]
            for bt in range(NBT):
                nc.scalar.activation(out=khe[bt], in_=psor[bt][:, M:M + 6],
                                     func=AF.Exp)
            uu, sp2, s3, kn2 = [], [], [], []
            for bt in range(NBT):
                u = sc("u", bt)
                nc.vector.tensor_scalar(out=u, in0=khe[bt][:, 1:2], scalar1=1.0,
                                        scalar2=None, op0=ALU.add)
                nc.vector.reciprocal(out=u, in_=u)          # u = 1-sig(g)
                uu.append(u)
                v2 = sc("sp2", bt, 2)
                nc.vector.tensor_scalar(out=v2, in0=_cols(khe[bt], 0, 5, 2),
                                        scalar1=1.0, scalar2=None, op0=ALU.add)
                sp2.append(v2)                              # [1+e^b, 1+e^gam]
                ssum = sc("ssum", bt)
                nc.vector.reduce_sum(out=ssum, in_=khe[bt][:, 2:5], axis=AX.X)
                nc.vector.reciprocal(out=ssum, in_=ssum)
                s3t = sc("s3", bt, 3)
                nc.vector.tensor_scalar(out=s3t, in0=khe[bt][:, 2:5],
                                        scalar1=ssum, scalar2=None, op0=ALU.mult)
                s3.append(s3t)
                kk = sc("kn2", bt)
                ksc = ppool.tile([128, M], FP, name=f"ksc_{sfx}_{bt}", tag="ksc",
                                 bufs=1)
                nc.vector.tensor_mul(ksc, kT[bt], kT[bt])
                nc.vector.reduce_sum(out=kk, in_=ksc, axis=AX.X)
                kn2.append(kk)

            # Ln block: softplus(beta/gamma) then ln(beta); ln(n2*k2)
            lsp, lnbeta, lden = [], [], []
            for bt in range(NBT):
                ls = sc("lsp", bt, 2)
                nc.scalar.activation(out=ls, in_=sp2[bt], func=AF.Ln)
                lsp.append(ls)                               # [beta, sp(gamma)]
            for bt in range(NBT):
                lb = sc("lnbeta", bt)
                nc.scalar.activation(out=lb, in_=lsp[bt][:, 0:1], func=AF.Ln)
                lnbeta.append(lb)
            for bt in range(NBT):
                n2k2 = apool.tile([128, N], FP, name=f"n2k2_{sfx}_{bt}",
                                  tag="n2k2", bufs=2)
                nc.vector.tensor_scalar(out=n2k2, in0=sqn[(t, bt)],
                                        scalar1=kn2[bt],
                                        scalar2=None, op0=ALU.mult)
                ld = apool.tile([128, N], FP, name=f"lden_{sfx}_{bt}",
                                tag="lden", bufs=2)
                nc.scalar.activation(out=ld, in_=n2k2, func=AF.Ln, bias=eps30)
                lden.append(ld)

            gam = []
            for bt in range(NBT):
                g = sc("gam", bt)
                nc.vector.tensor_scalar(out=g, in0=lsp[bt][:, 1:2], scalar1=1.0,
                                        scalar2=None, op0=ALU.add)
                gam.append(g)

            # cos numerator: mid-dim k broadcast runs in DVE 2x mode
            cn = []
            for bt in range(NBT):
                prod = ppool.tile([128, N, M], BF, name=f"prodc_{sfx}_{bt}",
                                  tag="prod", bufs=1)
                nc.vector.tensor_mul(prod, mem[(t, bt)],
                                     _bcast_mid(kT[bt], N))
                cnt = apool.tile([128, N], FP, name=f"cn_{sfx}_{bt}", tag="cn",
                                 bufs=2)
                tree_m(cnt, prod)
                cn.append(cnt)

            # Exp block: lw = exp(-0.5*ln(n2k2) + ln(beta)); e = exp(lw*cn)
            for bt in range(NBT):
                lw = apool.tile([128, N], FP, name=f"lw_{sfx}_{bt}", tag="lw",
                                bufs=2)
                nc.scalar.activation(out=lw, in_=lden[bt], func=AF.Exp,
                                     scale=-0.5, bias=lnbeta[bt])
                nc.vector.tensor_mul(cn[bt], cn[bt], lw)     # beta*cos logits
            ee = []
            for bt in range(NBT):
                e = apool.tile([128, N], FP, name=f"e_{sfx}_{bt}", tag="e",
                               bufs=2)
                nc.scalar.activation(out=e, in_=cn[bt], func=AF.Exp)
                ee.append(e)

            if stage < 45:
                for bt in range(NBT):
                    nc.sync.dma_start(
                        out=outT[t, bt * 128:(bt + 1) * 128, 0:N], in_=ee[bt])
                continue

            # wg = (g/sum_e)*e + (1-g)*w_prev, then 3-tap circular shift
            ws = []
            for bt in range(NBT):
                se = sc("se", bt)
                nc.vector.reduce_sum(out=se, in_=ee[bt], axis=AX.X)
                nc.vector.reciprocal(out=se, in_=se)
                gsig = sc("gsig", bt)
                nc.vector.tensor_scalar(out=gsig, in0=uu[bt], scalar1=-1.0,
                                        scalar2=1.0, op0=ALU.mult, op1=ALU.add)
                nc.vector.tensor_mul(gsig, gsig, se)         # g / sum_e
                w0p = apool.tile([128, N], FP, name=f"w0p_{sfx}_{bt}",
                                 tag="w0p", bufs=2)
                nc.vector.tensor_scalar(out=w0p, in0=w0[bt], scalar1=uu[bt],
                                        scalar2=None, op0=ALU.mult)
                wg = apool.tile([128, N], FP, name=f"wg_{sfx}_{bt}", tag="wg",
                                bufs=2)
                nc.vector.scalar_tensor_tensor(out=wg, in0=ee[bt],
                                               scalar=gsig, in1=w0p,
                                               op0=ALU.mult, op1=ALU.add)
                # ws = s0*roll(wg,+1) + s1*wg + s2*roll(wg,-1)
                wmid = apool.tile([128, N], FP, name=f"wmid_{sfx}_{bt}",
                                  tag="wmid", bufs=2)
                nc.vector.tensor_scalar(out=wmid, in0=wg, scalar1=s3[bt][:, 1:2],
                                        scalar2=None, op0=ALU.mult)
                wst = apool.tile([128, N], FP, name=f"ws_{sfx}_{bt}", tag="ws",
                                 bufs=2)
                nc.vector.scalar_tensor_tensor(out=wst[:, 1:N], in0=wg[:, 0:N - 1],
                                               scalar=s3[bt][:, 0:1],
                                               in1=wmid[:, 1:N],
                                               op0=ALU.mult, op1=ALU.add)
                nc.vector.scalar_tensor_tensor(out=wst[:, 0:1], in0=wg[:, N - 1:N],
                                               scalar=s3[bt][:, 0:1],
                                               in1=wmid[:, 0:1],
                                               op0=ALU.mult, op1=ALU.add)
                nc.vector.scalar_tensor_tensor(out=wmid[:, 0:N - 1],
                                               in0=wg[:, 1:N],
                                               scalar=s3[bt][:, 2:3],
                                               in1=wst[:, 0:N - 1],
                                               op0=ALU.mult, op1=ALU.add)
                nc.vector.scalar_tensor_tensor(out=wmid[:, N - 1:N],
                                               in0=wg[:, 0:1],
                                               scalar=s3[bt][:, 2:3],
                                               in1=wst[:, N - 1:N],
                                               op0=ALU.mult, op1=ALU.add)
                ws.append(wmid)

            # sharpen: wp = ws**gamma (unnormalised; fold 1/sum into r)
            lnws = []
            for bt in range(NBT):
                lt = apool.tile([128, N], FP, name=f"lnws_{sfx}_{bt}",
                                tag="lnws", bufs=2)
                nc.scalar.activation(out=lt, in_=ws[bt], func=AF.Ln, bias=eps30)
                nc.vector.tensor_scalar(out=lt, in0=lt, scalar1=gam[bt],
                                        scalar2=None, op0=ALU.mult)
                lnws.append(lt)
            wp = []
            for bt in range(NBT):
                wpt = apool.tile([128, N], BF, name=f"wp_{sfx}_{bt}", tag="wp",
                                 bufs=2)
                nc.scalar.activation(out=wpt, in_=lnws[bt], func=AF.Exp)
                wp.append(wpt)

            # r = sum_n wp*mem / sum_n wp
            for bt in range(NBT):
                bsl = slice(bt * 128, (bt + 1) * 128)
                swp = sc("swp", bt)
                nc.vector.reduce_sum(out=swp, in_=wp[bt], axis=AX.X)
                nc.vector.reciprocal(out=swp, in_=swp)
                prodr = ppool.tile([128, N, M], BF, name=f"prodr_{sfx}_{bt}",
                                   tag="prod", bufs=1)
                for nh in range(2):
                    nsl = slice(nh * (N // 2), (nh + 1) * (N // 2))
                    nc.vector.tensor_mul(prodr[:, nsl, :],
                                         mem[(t, bt)][:, nsl, :],
                                         _bcast_inner(wp[bt][:, nsl], M))
                rp = apool.tile([128, 1, M], FP, name=f"rp_{sfx}_{bt}",
                                tag="rp", bufs=2)
                tree_n(rp, prodr)
                rfin = apool.tile([128, M], FP, name=f"rfin_{sfx}_{bt}",
                                  tag="rfin", bufs=2)
                nc.vector.tensor_scalar(out=rfin, in0=rp[:, 0, :], scalar1=swp,
                                        scalar2=None, op0=ALU.mult)
                pst = mm_ps([M, 128], f"rtp_{sfx}_{bt}", tag="tp", bufs=1)
                nc.tensor.transpose(pst, rfin, ident)
                nc.vector.tensor_copy(out=rT_next[:, bsl], in_=pst)

            if stage < 99:
                nc.sync.dma_start(out=outT[t, 0:M, :], in_=rT_next)
                rT_prev = rT_next
                continue

            # output projection of this t is deferred into iteration t+1
            pending_out = (t, wo, h, rT_next, boc)
            rT_prev = rT_next

        if pending_out is not None and stage >= 99:
            emit_out(*pending_out)

    nc.compile()
    return nc


_CACHE = {}
LAST = {}


def _get_nc():
    if "nc" not in _CACHE:
        _CACHE["nc"] = build_nc()
    return _CACHE["nc"]


def host_prep(inputs, W1, b1, lng, lnb, W2, b2, Wih, Whh, bih, bhh,
              Wr, br, Ww, bw, Wo, bo, mem0, read0, wr0, ww0, h0, c0):
    f32 = np.float32
    inputs, W1, W2, Wih, Whh, Wr, Wo = [np.asarray(a, f32) for a in
                                        (inputs, W1, W2, Wih, Whh, Wr, Wo)]

    def percol(v, cols):   # [T, 128*cols] -> [T, 128, cols] column-major chunks
        return np.ascontiguousarray(
            np.asarray(v, f32).reshape(T, cols, 128).transpose(0, 2, 1))

    bf = ml_dtypes.bfloat16
    xT_full = np.ascontiguousarray(inputs.transpose(0, 2, 1)).astype(bf)
    w1t = np.ascontiguousarray(W1.transpose(0, 2, 1)).astype(bf)   # [T, E, H]
    w2t = np.ascontiguousarray(W2.transpose(0, 2, 1)).astype(bf)   # [T, H, V]
    wiht = np.ascontiguousarray(Wih.transpose(0, 2, 1)).astype(bf)
    whht = np.ascontiguousarray(Whh.transpose(0, 2, 1)).astype(bf)
    wrt = np.ascontiguousarray(Wr.transpose(0, 2, 1)).astype(bf)   # [T, H, 70]
    wot = np.ascontiguousarray(Wo.transpose(0, 2, 1)).astype(bf)   # [T, 576, E]
    h0t_full = np.asarray(h0, f32).transpose(0, 2, 1).astype(bf)
    c0t_full = np.asarray(c0, f32).transpose(0, 2, 1).astype(bf)
    r0t_full = np.asarray(read0, f32)[T - 1].T.astype(bf)          # [M, B]
    wr0_full = np.asarray(wr0, f32).astype(bf)
    mem0_full = np.asarray(mem0).astype(bf)
    bz = np.asarray(bih, f32) + np.asarray(bhh, f32)

    common = dict(
        w1t=w1t, w2t=w2t, wiht=wiht, whht=whht, wrt=wrt, wot=wot,
        b1c=percol(b1, HC), lngc=percol(lng, HC), lnbc=percol(lnb, HC),
        b2c=percol(b2, VC), bzc=percol(bz, ZC),
        brr=np.ascontiguousarray(np.asarray(br, f32).reshape(T, 1, M + 6)),
        boc=percol(bo, EC),
    )
    in_maps = []
    for ci in range(NCORES):
        bsl = slice(ci * BS, (ci + 1) * BS)
        in_maps.append(dict(
            common,
            xT=np.ascontiguousarray(xT_full[:, :, bsl]),
            h0t=np.ascontiguousarray(h0t_full[:, :, bsl]),
            c0t=np.ascontiguousarray(c0t_full[:, :, bsl]),
            r0t=np.ascontiguousarray(r0t_full[:, bsl]),
            wr0=np.ascontiguousarray(wr0_full[:, bsl, :]),
            mem0=np.ascontiguousarray(mem0_full[:, bsl]),
        ))

    return in_maps


def kernel(**inputs):
    in_maps = host_prep(**inputs)
    nc = _get_nc()
    import os
    trace = os.environ.get("BASS_TRACE", "") not in ("", "0")
    res = run_bass_kernel_spmd(nc, in_maps, list(range(NCORES)), trace=trace)
    LAST["exec_time_ns"] = res.exec_time_ns
    LAST["results"] = res
    out = np.concatenate(
        [np.transpose(r["outT"], (0, 2, 1)) for r in res.results], axis=1)
    return np.ascontiguousarray(out.astype(np.float32))


# revision 40
# speedup vs baseline: 1.0157x; 1.0098x over previous
"""Trainium2 Bass kernel for nn_CM_NTM_29566554866014 (scatter_memory).

Sharding: pure batch data-parallelism across 8 NeuronCores (B=2048 -> 256/core).
Small parameters replicated. The cross-NTM loop (T=4) is sequential but
batch-local, so each core runs all 4 steps on its batch shard independently.
No collectives.

Key structural facts used (verified against the reference math):
  * The write head (Ww/bw/ww0) and the memory erase/add update are dead code:
    `mem` is reassigned to `mem0[i+1]` each iteration and outputs depend only
    on h and r. They are therefore not computed.
  * Only read0[T-1] is consumed.
  * Per-step state (mem0/h0/c0/wr0) are fresh inputs each step; the only
    sequential dependency across steps is the read vector r.

Engine assignment (v5, evolved via perfetto traces; 524us -> ~426us):
  * DVE (vector) is the bottleneck engine; every big elementwise op is bf16
    so it runs in DVE 2x mode. Empirical 2x rule on TRN2: a stride-0
    broadcast on a MIDDLE free dim keeps 2x (cos numerator: k broadcast
    over n), a stride-0 broadcast on the INNERMOST dim drops to 1x (read
    weighting: w broadcast over m - unavoidable in this layout).
  * gpsimd is a trap here: it shares SBUF read/write ports with the DVE,
    so offloading big elementwise ops to it inflates concurrent DVE ops
    3-5x. All elementwise stays on DVE/ACT.
  * mem row-norm squares run on the ACT engine (Square), grouped with the
    LN squares into one table visit; activation calls are grouped per
    function to minimise ACT table reloads (1.28us each).
  * addressing math is Exp/Ln-only: 1/(|m||k|) and *beta folded into one
    Exp(-0.5*ln(n2*k2) + ln(beta)) ACT op; softmax max-subtraction dropped
    (logits bounded); softplus via shared Exp/Ln blocks; sharpen left
    unnormalised and the 1/sum folded into the read vector.
  * read-head output computed transposed on the PE (h as stationary), which
    removes the k/kh PE transposes + drains; bias via a ones-row matmul.
  * output projection of step t is deferred into step t+1 so the PE runs
    proj(t+1) while the DVE walks t's addressing chain.
  * tensor_tensor_reduce crashes the HW runtime (NRT_EXEC_UNIT_UNRECOVERABLE)
    - do not use it.
"""

import numpy as np
import ml_dtypes
from contextlib import ExitStack

import concourse.bass as bass
import concourse.tile as tile
from concourse import bacc
from concourse import mybir
from concourse.bass_utils import run_bass_kernel_spmd
from concourse.masks import make_identity

AF = mybir.ActivationFunctionType
ALU = mybir.AluOpType
AX = mybir.AxisListType
FP = mybir.dt.float32
BF = mybir.dt.bfloat16

T, E, V, H, N, M, B = 4, 512, 256, 512, 128, 64, 2048
NCORES = 8
BS = B // NCORES      # 256 batch rows per core
NBT = BS // 128       # 2 batch tiles
HC = H // 128         # 4
EC = E // 128         # 4
VC = V // 128         # 2
ZC = (4 * H) // 128   # 16


def _bcast_inner(ap, count):
    """View `ap` ([P, F]) as [P, F, count] with a stride-0 innermost dim."""
    return bass.AP(tensor=ap.tensor, offset=ap.offset,
                   ap=[*ap.ap, [0, count]])


def _bcast_mid(ap, count):
    """View `ap` ([P, F]) as [P, count, F] with a stride-0 middle dim."""
    return bass.AP(tensor=ap.tensor, offset=ap.offset,
                   ap=[ap.ap[0], [0, count], ap.ap[1]])


def _cols(ap, start, step, count):
    """Strided column view of a 2-dim AP: columns start, start+step, ..."""
    st = ap.ap[1][0]
    return bass.AP(tensor=ap.tensor, offset=ap.offset + start * st,
                   ap=[ap.ap[0], [step * st, count]])


def build_nc(stage=None):
    import os
    if stage is None:
        stage = int(os.environ.get("NTM_STAGE", "99"))
    nc = bacc.Bacc()
    d = {}

    def din(name, shape, dt=FP):
        d[name] = nc.dram_tensor(name, list(shape), dt, kind="ExternalInput")

    din("xT",   (T, E, BS), BF)
    din("w1t",  (T, E, H), BF)
    din("w2t",  (T, H, V), BF)
    din("wiht", (T, V + M, 4 * H), BF)
    din("whht", (T, H, 4 * H), BF)
    din("wrt",  (T, H, M + 6), BF)
    din("wot",  (T, H + M, E), BF)
    din("h0t",  (T, H, BS), BF)
    din("c0t",  (T, H, BS), BF)
    din("r0t",  (M, BS), BF)
    din("wr0",  (T, BS, N), BF)
    din("mem0", (T, BS, N, M), BF)
    din("b1c",  (T, 128, HC))
    din("lngc", (T, 128, HC))
    din("lnbc", (T, 128, HC))
    din("b2c",  (T, 128, VC))
    din("bzc",  (T, 128, ZC))
    din("brr",  (T, 1, M + 6))
    din("boc",  (T, 128, EC))
    outT = nc.dram_tensor("outT", [T, E, BS], FP, kind="ExternalOutput")

    with tile.TileContext(nc) as tc, ExitStack() as ctx:
        singles = ctx.enter_context(tc.tile_pool(name="singles", bufs=1))
        wpool = ctx.enter_context(tc.tile_pool(name="wpool", bufs=1))
        spool = ctx.enter_context(tc.tile_pool(name="spool", bufs=1))
        apool = ctx.enter_context(tc.tile_pool(name="apool", bufs=1))
        mpool = ctx.enter_context(tc.tile_pool(name="mpool", bufs=1))
        ppool = ctx.enter_context(tc.tile_pool(name="ppool", bufs=1))
        pmm = ctx.enter_context(tc.tile_pool(name="pmm", bufs=1, space="PSUM"))

        ones_t = singles.tile([128, 128], FP, name="ones_t")
        nc.vector.memset(ones_t, 1.0)
        ones_row = singles.tile([1, 128], FP, name="ones_row")
        nc.vector.memset(ones_row, 1.0)
        ones_b = singles.tile([128, 128], BF, name="ones_b")
        nc.vector.memset(ones_b, 1.0)
        ident = singles.tile([128, 128], FP, name="ident")
        make_identity(nc, ident)
        eps30 = singles.tile([128, 1], FP, name="eps30")
        nc.vector.memset(eps30, 1e-30)

        def mm_ps(shape, name, tag="mm", bufs=5):
            return pmm.tile(shape, FP, name=name, tag=tag, bufs=bufs)

        def tree_m(dst2d, prod, eng=None):
            """Sum prod [128, N, M] over innermost m into dst2d [128, N] fp32
            via pairwise bf16 adds (DVE 2x mode)."""
            eng = eng or nc.vector
            G = prod.shape[1]
            s1 = ppool.tile([128, G, M // 2], BF, name="trm", tag="trm", bufs=1)
            eng.tensor_add(s1, prod[:, :, 0:M // 2], prod[:, :, M // 2:M])
            w = M // 2
            while w > 2:
                hw = w // 2
                eng.tensor_add(s1[:, :, 0:hw], s1[:, :, 0:hw], s1[:, :, hw:w])
                w = hw
            dst3 = bass.AP(tensor=dst2d.tensor, offset=dst2d.offset,
                           ap=[*dst2d.ap, [1, 1]])
            eng.tensor_add(dst3, s1[:, :, 0:1], s1[:, :, 1:2])

        def tree_n(dst3d, prod):
            """Sum prod [128, N, M] over mid n into dst3d [128, 1, M] fp32
            via pairwise bf16 adds on contiguous halves."""
            G = prod.shape[1]
            s1 = ppool.tile([128, G // 2, M], BF, name="trn", tag="trn", bufs=1)
            nc.vector.tensor_add(s1, prod[:, 0:G // 2, :], prod[:, G // 2:G, :])
            w = G // 2
            while w > 2:
                hw = w // 2
                nc.vector.tensor_add(s1[:, 0:hw, :], s1[:, 0:hw, :],
                                     s1[:, hw:w, :])
                w = hw
            nc.vector.tensor_add(dst3d, s1[:, 0:1, :], s1[:, 1:2, :])

        mem = {}
        sqn = {}
        sqp = {}

        def emit_sq(tn, bt):
            pr = ppool.tile([128, N, M], BF, name=f"sqp_t{tn}_{bt}",
                            tag="sqp", bufs=2)
            nc.scalar.square(pr, mem[(tn, bt)])
            sqp[(tn, bt)] = pr

        def emit_tree(tn, bt):
            n2 = apool.tile([128, N], FP, name=f"n2_t{tn}_{bt}", tag="n2",
                            bufs=4)
            tree_m(n2, sqp[(tn, bt)])
            sqn[(tn, bt)] = n2

        def load_mem(t):
            for bt in range(NBT):
                mt = mpool.tile([128, N, M], BF, name=f"mem_t{t}_{bt}",
                                tag="mem", bufs=3)
                nc.sync.dma_start(out=mt, in_=d["mem0"][t, bt * 128:(bt + 1) * 128])
                mem[(t, bt)] = mt

        def emit_out(to, wo_, h_, rT_, boc_):
            for ec in range(EC):
                esl = slice(ec * 128, (ec + 1) * 128)
                ps = mm_ps([128, BS], f"o_t{to}_{ec}")
                for k in range(4):
                    nc.tensor.matmul(ps, wo_[k][:, esl], h_[k], start=(k == 0),
                                     stop=False)
                nc.tensor.matmul(ps, wo_[4][:, esl], rT_, start=False,
                                 stop=True)
                os_ = apool.tile([128, BS], FP, name=f"os_t{to}_{ec}", tag="os",
                                 bufs=2)
                nc.scalar.activation(out=os_, in_=ps, func=AF.Sigmoid,
                                     bias=boc_[:, ec:ec + 1])
                nc.sync.dma_start(out=outT[to, esl, :], in_=os_)

        pending_out = None
        rT_prev = None
        for t in range(T):
            sfx = f"t{t}"
            # ---------------- loads ----------------
            # Issue order matters: the sync engine fires DMAs in program
            # order, so load what this t needs first (proj inputs, mem for
            # the ACT squares), then the late-phase weights, then prefetch.
            w1 = [wpool.tile([128, H], BF, name=f"w1_{sfx}_{k}", tag="w1",
                             bufs=4) for k in range(4)]
            for k in range(4):
                nc.sync.dma_start(out=w1[k], in_=d["w1t"][t, k * 128:(k + 1) * 128, :])
            xT = [spool.tile([128, BS], BF, name=f"xT_{sfx}_{k}", tag="xT",
                             bufs=4) for k in range(4)]
            for k in range(4):
                nc.sync.dma_start(out=xT[k], in_=d["xT"][t, k * 128:(k + 1) * 128, :])
            b1c = spool.tile([128, HC], FP, name=f"b1c_{sfx}", tag="b1c", bufs=2)
            lng = spool.tile([128, HC], FP, name=f"lng_{sfx}", tag="lng", bufs=2)
            lnb = spool.tile([128, HC], FP, name=f"lnb_{sfx}", tag="lnb", bufs=2)
            b2c = spool.tile([128, VC], FP, name=f"b2c_{sfx}", tag="b2c", bufs=2)
            bzc = spool.tile([128, ZC], FP, name=f"bzc_{sfx}", tag="bzc", bufs=2)
            brr = spool.tile([1, M + 6], FP, name=f"brr_{sfx}", tag="brr", bufs=2)
            boc = spool.tile([128, EC], FP, name=f"boc_{sfx}", tag="boc", bufs=2)
            nc.sync.dma_start(out=b1c, in_=d["b1c"][t])
            nc.sync.dma_start(out=lng, in_=d["lngc"][t])
            nc.sync.dma_start(out=lnb, in_=d["lnbc"][t])
            nc.sync.dma_start(out=b2c, in_=d["b2c"][t])
            if t == 0:
                load_mem(0)
            w2 = [wpool.tile([128, V], BF, name=f"w2_{sfx}_{k}", tag="w2",
                             bufs=4) for k in range(4)]
            for k in range(4):
                nc.sync.dma_start(out=w2[k], in_=d["w2t"][t, k * 128:(k + 1) * 128, :])
            h0 = [spool.tile([128, BS], BF, name=f"h0_{sfx}_{k}", tag="h0",
                             bufs=4) for k in range(4)]
            c0b = spool.tile([128, HC, BS], BF, name=f"c0b_{sfx}", tag="c0",
                             bufs=2)
            for k in range(4):
                nc.sync.dma_start(out=h0[k], in_=d["h0t"][t, k * 128:(k + 1) * 128, :])
                nc.sync.dma_start(out=c0b[:, k, :],
                                  in_=d["c0t"][t, k * 128:(k + 1) * 128, :])
            nc.sync.dma_start(out=bzc, in_=d["bzc"][t])
            wih = []
            for k, ksz in enumerate((128, 128, 64)):
                wt = wpool.tile([ksz, 4 * H], BF, name=f"wih_{sfx}_{k}", tag="wih",
                                bufs=3)
                nc.sync.dma_start(out=wt, in_=d["wiht"][t, k * 128:k * 128 + ksz, :])
                wih.append(wt)
            whh = [wpool.tile([128, 4 * H], BF, name=f"whh_{sfx}_{k}", tag="whh",
                              bufs=4) for k in range(4)]
            for k in range(4):
                nc.sync.dma_start(out=whh[k], in_=d["whht"][t, k * 128:(k + 1) * 128, :])
            wr_ = [wpool.tile([128, M + 6], BF, name=f"wr_{sfx}_{k}", tag="wr",
                              bufs=4) for k in range(4)]
            for k in range(4):
                nc.sync.dma_start(out=wr_[k], in_=d["wrt"][t, k * 128:(k + 1) * 128, :])
            nc.sync.dma_start(out=brr, in_=d["brr"][t])
            w0 = []
            for bt in range(NBT):
                wt = spool.tile([128, N], BF, name=f"w0_{sfx}_{bt}", tag="w0",
                                bufs=4)
                nc.sync.dma_start(out=wt, in_=d["wr0"][t, bt * 128:(bt + 1) * 128, :])
                w0.append(wt)
            wo = []
            for k, ksz in enumerate((128, 128, 128, 128, 64)):
                wt = wpool.tile([ksz, E], BF, name=f"wo_{sfx}_{k}", tag="wo", bufs=5)
                nc.sync.dma_start(out=wt, in_=d["wot"][t, k * 128:k * 128 + ksz, :])
                wo.append(wt)
            nc.sync.dma_start(out=boc, in_=d["boc"][t])
            if t + 1 < T:
                load_mem(t + 1)

            if t == 0:
                rT_prev = spool.tile([M, BS], BF, name="r0T", tag="rT", bufs=2)
                nc.sync.dma_start(out=rT_prev, in_=d["r0t"][:, :])

            # ---------------- input projection + LN + p ----------------
            a1 = apool.tile([128, HC, BS], FP, name=f"a1_{sfx}", tag="a1", bufs=1)
            for hc in range(HC):
                ps = mm_ps([128, BS], f"a1_{sfx}_{hc}")
                for k in range(4):
                    nc.tensor.matmul(ps, w1[k][:, hc * 128:(hc + 1) * 128], xT[k],
                                     start=(k == 0), stop=(k == 3))
                nc.vector.tensor_scalar(out=a1[:, hc, :], in0=ps,
                                        scalar1=b1c[:, hc:hc + 1], scalar2=None,
                                        op0=ALU.add)

            ps_sum = mm_ps([128, BS], f"sums_{sfx}")
            for k in range(4):
                nc.tensor.matmul(ps_sum, ones_t, a1[:, k, :], start=(k == 0),
                                 stop=(k == 3))
            # -------- Square block: mem row squares + LN squares (one ACT
            # table visit per t; Sqrt/Relu follow adjacently). bt0 of this
            # t was already emitted one step ahead. --------
            for bt in range(NBT):
                if (t, bt) not in sqp:
                    emit_sq(t, bt)
            sq4 = apool.tile([128, HC, BS], BF, name=f"sq4_{sfx}", tag="sq4",
                             bufs=1)
            for k in range(4):
                nc.scalar.square(sq4[:, k, :], a1[:, k, :])
            ps_sq = mm_ps([128, BS], f"sumsq_{sfx}")
            for k in range(4):
                nc.tensor.matmul(ps_sq, ones_b, sq4[:, k, :], start=(k == 0),
                                 stop=(k == 3))

            mu = apool.tile([128, BS], FP, name=f"mu_{sfx}", tag="mu", bufs=1)
            nc.vector.tensor_scalar(out=mu, in0=ps_sum, scalar1=1.0 / H,
                                    scalar2=None, op0=ALU.mult)
            var = apool.tile([128, BS], FP, name=f"var_{sfx}", tag="var", bufs=1)
            nc.vector.tensor_mul(var, mu, mu)
            nc.vector.scalar_tensor_tensor(out=var, in0=ps_sq, scalar=1.0 / H,
                                           in1=var, op0=ALU.mult,
                                           op1=ALU.subtract)
            nc.vector.tensor_scalar(out=var, in0=var, scalar1=1e-5,
                                    scalar2=None, op0=ALU.add)
            nc.vector.reciprocal(out=var, in_=var)
            rstd = apool.tile([128, BS], FP, name=f"rstd_{sfx}", tag="rstd",
                              bufs=1)
            nc.scalar.activation(out=rstd, in_=var, func=AF.Sqrt)

            nc.vector.tensor_sub(a1, a1, _bcast_mid(mu, HC))
            nc.vector.tensor_mul(a1, a1, _bcast_mid(rstd, HC))
            lnt = []
            for hc in range(HC):
                lt = apool.tile([128, BS], BF, name=f"lnt_{sfx}_{hc}", tag="lnt",
                                bufs=4)
                nc.scalar.activation(out=lt, in_=a1[:, hc, :], func=AF.Relu,
                                     bias=lnb[:, hc:hc + 1],
                                     scale=lng[:, hc:hc + 1])
                lnt.append(lt)

            p = []
            for vc in range(VC):
                ps = mm_ps([128, BS], f"p_{sfx}_{vc}")
                for k in range(4):
                    nc.tensor.matmul(ps, w2[k][:, vc * 128:(vc + 1) * 128], lnt[k],
                                     start=(k == 0), stop=(k == 3))
                pt = apool.tile([128, BS], BF, name=f"p_{sfx}_{vc}", tag="p", bufs=2)
                nc.scalar.activation(out=pt, in_=ps, func=AF.Tanh,
                                     bias=b2c[:, vc:vc + 1])
                p.append(pt)

            if stage < 2:
                for vc in range(VC):
                    nc.sync.dma_start(out=outT[t, vc * 128:(vc + 1) * 128, :],
                                      in_=p[vc])
                continue

            # -------- mem row norms: trees fill the gate-phase DVE bubble.
            # The bt0 square+tree of t+1 is emitted here too (its mem buffer
            # recycles from t-1, so no dependency cycle) --------
            for bt in range(NBT):
                if (t, bt) not in sqn:
                    emit_tree(t, bt)
            if t + 1 < T:
                emit_sq(t + 1, 0)
                emit_tree(t + 1, 0)

            # ---- deferred output projection of t-1 (keeps PE busy during
            # t-1's addressing chain; its sigma-ACTs join t's sigmoid block) --
            if pending_out is not None:
                emit_out(*pending_out)
                pending_out = None

            # ---------------- LSTM (chain starts: needs rT_prev) ----------------
            def z_chain(oc):
                osl = slice(oc * 128, (oc + 1) * 128)
                ps = mm_ps([128, BS], f"z_{sfx}_{oc}")
                nc.tensor.matmul(ps, wih[0][:, osl], p[0], start=True, stop=False)
                nc.tensor.matmul(ps, wih[1][:, osl], p[1], start=False, stop=False)
                for k in range(4):
                    nc.tensor.matmul(ps, whh[k][:, osl], h0[k], start=False,
                                     stop=False)
                nc.tensor.matmul(ps, wih[2][:, osl], rT_prev, start=False,
                                 stop=True)
                return ps

            gates = {}
            for hc in range(HC):
                for gi in (0, 1, 3):          # sigmoid gates grouped
                    oc = gi * 4 + hc
                    ps = z_chain(oc)
                    gs = apool.tile([128, BS], BF, name=f"g_{sfx}_{oc}", tag="gt",
                                    bufs=16)
                    nc.scalar.activation(out=gs, in_=ps, func=AF.Sigmoid,
                                         bias=bzc[:, oc:oc + 1])
                    gates[(gi, hc)] = gs
            for hc in range(HC):              # tanh gates grouped
                oc = 2 * 4 + hc
                ps = z_chain(oc)
                gs = apool.tile([128, BS], BF, name=f"g_{sfx}_{oc}", tag="gt",
                                bufs=16)
                nc.scalar.activation(out=gs, in_=ps, func=AF.Tanh,
                                     bias=bzc[:, oc:oc + 1])
                gates[(2, hc)] = gs

            cc = []
            for hc in range(HC):
                t2 = apool.tile([128, BS], BF, name=f"ct2_{sfx}_{hc}", tag="ct",
                                bufs=4)
                nc.vector.tensor_mul(t2, gates[(0, hc)], gates[(2, hc)])
                gf = gates[(1, hc)]
                nc.vector.tensor_mul(gf, gf, c0b[:, hc, :])  # gf = f*c0
                nc.vector.tensor_add(t2, t2, gf)            # t2 = c
                cc.append(t2)
            tc_ = []
            for hc in range(HC):
                th = apool.tile([128, BS], BF, name=f"tc_{sfx}_{hc}", tag="tch",
                                bufs=4)
                nc.scalar.activation(out=th, in_=cc[hc], func=AF.Tanh)
                tc_.append(th)
            h = []
            for hc in range(HC):
                ht = apool.tile([128, BS], BF, name=f"h_{sfx}_{hc}", tag="h",
                                bufs=4)
                nc.vector.tensor_mul(ht, gates[(3, hc)], tc_[hc])
                h.append(ht)

            if stage < 3:
                for k in range(4):
                    nc.sync.dma_start(out=outT[t, k * 128:(k + 1) * 128, :],
                                      in_=h[k])
                continue

            # ---------------- read head: or^T = h^T Wr^T + br ----------------
            # PE with h (stationary) x wr (moving) gives [batch, M+6] directly.
            kT = []
            psor = []
            for bt in range(NBT):
                bsl = slice(bt * 128, (bt + 1) * 128)
                ps = mm_ps([128, M + 6], f"orT_{sfx}_{bt}", tag="orT", bufs=2)
                for k in range(4):
                    nc.tensor.matmul(ps, h[k][:, bsl], wr_[k], start=(k == 0),
                                     stop=False)
                nc.tensor.matmul(ps, ones_row, brr, start=False, stop=True)
                kt = apool.tile([128, M], BF, name=f"kT_{sfx}_{bt}", tag="kT",
                                bufs=2)
                nc.scalar.activation(out=kt, in_=ps[:, 0:M], func=AF.Tanh)
                kT.append(kt)
                psor.append(ps)

            rT_next = spool.tile([M, BS], BF, name=f"rT_{sfx}", tag="rT", bufs=2)

            # ---- addressing, bt-batched per ACT-table stage ----
            def sc(nm, bt, w=1):
                return apool.tile([128, w], FP, name=f"{nm}_{sfx}_{bt}",
                                  tag="sc1", bufs=24)

            # Exp of raw head scalars [beta, g, s0, s1, s2, gamma]
            khe = [sc("khe", bt, 6) for bt in range(NBT)]
            for bt in range(NBT):
                nc.scalar.activation(out=khe[bt], in_=psor[bt][:, M:M + 6],
                                     func=AF.Exp)
            uu, sp2, s3, kn2 = [], [], [], []
            for bt in range(NBT):
                u = sc("u", bt)
                nc.vector.tensor_scalar(out=u, in0=khe[bt][:, 1:2], scalar1=1.0,
                                        scalar2=None, op0=ALU.add)
                nc.vector.reciprocal(out=u, in_=u)          # u = 1-sig(g)
                uu.append(u)
                v2 = sc("sp2", bt, 2)
                nc.vector.tensor_scalar(out=v2, in0=_cols(khe[bt], 0, 5, 2),
                                        scalar1=1.0, scalar2=None, op0=ALU.add)
                sp2.append(v2)                              # [1+e^b, 1+e^gam]
                ssum = sc("ssum", bt)
                nc.vector.reduce_sum(out=ssum, in_=khe[bt][:, 2:5], axis=AX.X)
                nc.vector.reciprocal(out=ssum, in_=ssum)
                s3t = sc("s3", bt, 3)
                nc.vector.tensor_scalar(out=s3t, in0=khe[bt][:, 2:5],
                                        scalar1=ssum, scalar2=None, op0=ALU.mult)
                s3.append(s3t)
                kk = sc("kn2", bt)
                ksc = ppool.tile([128, M], FP, name=f"ksc_{sfx}_{bt}", tag="ksc",
                                 bufs=1)
                nc.vector.tensor_mul(ksc, kT[bt], kT[bt])
                nc.vector.reduce_sum(out=kk, in_=ksc, axis=AX.X)
                kn2.append(kk)

            # Ln block: softplus(beta/gamma) then ln(beta); ln(n2*k2)
            lsp, lnbeta, lden = [], [], []
            for bt in range(NBT):
                ls = sc("lsp", bt, 2)
                nc.scalar.activation(out=ls, in_=sp2[bt], func=AF.Ln)
                lsp.append(ls)                               # [beta, sp(gamma)]
            for bt in range(NBT):
                lb = sc("lnbeta", bt)
                nc.scalar.activation(out=lb, in_=lsp[bt][:, 0:1], func=AF.Ln)
                lnbeta.append(lb)
            for bt in range(NBT):
                n2k2 = apool.tile([128, N], FP, name=f"n2k2_{sfx}_{bt}",
                                  tag="n2k2", bufs=2)
                nc.vector.tensor_scalar(out=n2k2, in0=sqn[(t, bt)],
                                        scalar1=kn2[bt],
                                        scalar2=None, op0=ALU.mult)
                ld = apool.tile([128, N], FP, name=f"lden_{sfx}_{bt}",
                                tag="lden", bufs=2)
                nc.scalar.activation(out=ld, in_=n2k2, func=AF.Ln, bias=eps30)
                lden.append(ld)

            gam = []
            for bt in range(NBT):
                g = sc("gam", bt)
                nc.vector.tensor_scalar(out=g, in0=lsp[bt][:, 1:2], scalar1=1.0,
                                        scalar2=None, op0=ALU.add)
                gam.append(g)

            # cos numerator: mid-dim k broadcast runs in DVE 2x mode
            cn = []
            for bt in range(NBT):
                prod = ppool.tile([128, N, M], BF, name=f"prodc_{sfx}_{bt}",
                                  tag="prod", bufs=1)
                nc.vector.tensor_mul(prod, mem[(t, bt)],
                                     _bcast_mid(kT[bt], N))
                cnt = apool.tile([128, N], FP, name=f"cn_{sfx}_{bt}", tag="cn",
                                 bufs=2)
                tree_m(cnt, prod)
                cn.append(cnt)

            # Exp block: lw = exp(-0.5*ln(n2k2) + ln(beta)); e = exp(lw*cn)
            for bt in range(NBT):
                lw = apool.tile([128, N], FP, name=f"lw_{sfx}_{bt}", tag="lw",
                                bufs=2)
                nc.scalar.activation(out=lw, in_=lden[bt], func=AF.Exp,
                                     scale=-0.5, bias=lnbeta[bt])
                nc.vector.tensor_mul(cn[bt], cn[bt], lw)     # beta*cos logits
            ee = []
            for bt in range(NBT):
                e = apool.tile([128, N], FP, name=f"e_{sfx}_{bt}", tag="e",
                               bufs=2)
                nc.scalar.activation(out=e, in_=cn[bt], func=AF.Exp)
                ee.append(e)

            if stage < 45:
                for bt in range(NBT):
                    nc.sync.dma_start(
                        out=outT[t, bt * 128:(bt + 1) * 128, 0:N], in_=ee[bt])
                continue

            # wg = (g/sum_e)*e + (1-g)*w_prev, then 3-tap circular shift
            ws = []
            for bt in range(NBT):
                se = sc("se", bt)
                nc.vector.reduce_sum(out=se, in_=ee[bt], axis=AX.X)
                nc.vector.reciprocal(out=se, in_=se)
                gsig = sc("gsig", bt)
                nc.vector.tensor_scalar(out=gsig, in0=uu[bt], scalar1=-1.0,
                                        scalar2=1.0, op0=ALU.mult, op1=ALU.add)
                nc.vector.tensor_mul(gsig, gsig, se)         # g / sum_e
                w0p = apool.tile([128, N], FP, name=f"w0p_{sfx}_{bt}",
                                 tag="w0p", bufs=2)
                nc.vector.tensor_scalar(out=w0p, in0=w0[bt], scalar1=uu[bt],
                                        scalar2=None, op0=ALU.mult)
                wg = apool.tile([128, N], FP, name=f"wg_{sfx}_{bt}", tag="wg",
                                bufs=2)
                nc.vector.scalar_tensor_tensor(out=wg, in0=ee[bt],
                                               scalar=gsig, in1=w0p,
                                               op0=ALU.mult, op1=ALU.add)
                # ws = s0*roll(wg,+1) + s1*wg + s2*roll(wg,-1)
                wmid = apool.tile([128, N], FP, name=f"wmid_{sfx}_{bt}",
                                  tag="wmid", bufs=2)
                nc.vector.tensor_scalar(out=wmid, in0=wg, scalar1=s3[bt][:, 1:2],
                                        scalar2=None, op0=ALU.mult)
                wst = apool.tile([128, N], FP, name=f"ws_{sfx}_{bt}", tag="ws",
                                 bufs=2)
                nc.vector.scalar_tensor_tensor(out=wst[:, 1:N], in0=wg[:, 0:N - 1],
                                               scalar=s3[bt][:, 0:1],
                                               in1=wmid[:, 1:N],
                                               op0=ALU.mult, op1=ALU.add)
                nc.vector.scalar_tensor_tensor(out=wst[:, 0:1], in0=wg[:, N - 1:N],
                                               scalar=s3[bt][:, 0:1],
                                               in1=wmid[:, 0:1],
                                               op0=ALU.mult, op1=ALU.add)
                nc.vector.scalar_tensor_tensor(out=wmid[:, 0:N - 1],
                                               in0=wg[:, 1:N],
                                               scalar=s3[bt][:, 2:3],
                                               in1=wst[:, 0:N - 1],
                                               op0=ALU.mult, op1=ALU.add)
                nc.vector.scalar_tensor_tensor(out=wmid[:, N - 1:N],
                                               in0=wg[:, 0:1],
                                               scalar=s3[bt][:, 2:3],
                                               in1=wst[:, N - 1:N],
                                               op0=ALU.mult, op1=ALU.add)
                ws.append(wmid)

            # sharpen: wp = ws**gamma (unnormalised; fold 1/sum into r)
            lnws = []
            for bt in range(NBT):
                lt = apool.tile([128, N], FP, name=f"lnws_{sfx}_{bt}",
                                tag="lnws", bufs=2)
                nc.scalar.activation(out=lt, in_=ws[bt], func=AF.Ln, bias=eps30)
                nc.vector.tensor_scalar(out=lt, in0=lt, scalar1=gam[bt],
                                        scalar2=None, op0=ALU.mult)
                lnws.append(lt)
            wp = []
            for bt in range(NBT):
                wpt = apool.tile([128, N], BF, name=f"wp_{sfx}_{bt}", tag="wp",
                                 bufs=2)
                nc.scalar.activation(out=wpt, in_=lnws[bt], func=AF.Exp)
                wp.append(wpt)

            # r = sum_n wp*mem / sum_n wp
            for bt in range(NBT):
                bsl = slice(bt * 128, (bt + 1) * 128)
                swp = sc("swp", bt)
                nc.vector.reduce_sum(out=swp, in_=wp[bt], axis=AX.X)
                nc.vector.reciprocal(out=swp, in_=swp)
                prodr = ppool.tile([128, N, M], BF, name=f"prodr_{sfx}_{bt}",
                                   tag="prod", bufs=1)
                for nh in range(2):
                    nsl = slice(nh * (N // 2), (nh + 1) * (N // 2))
                    nc.vector.tensor_mul(prodr[:, nsl, :],
                                         mem[(t, bt)][:, nsl, :],
                                         _bcast_inner(wp[bt][:, nsl], M))
                rp = apool.tile([128, 1, M], FP, name=f"rp_{sfx}_{bt}",
                                tag="rp", bufs=2)
                tree_n(rp, prodr)
                rfin = apool.tile([128, M], FP, name=f"rfin_{sfx}_{bt}",
                                  tag="rfin", bufs=2)
                nc.vector.tensor_scalar(out=rfin, in0=rp[:, 0, :], scalar1=swp,
                                        scalar2=None, op0=ALU.mult)
                pst = mm_ps([M, 128], f"rtp_{sfx}_{bt}", tag="tp", bufs=1)
                nc.tensor.transpose(pst, rfin, ident)
                nc.vector.tensor_copy(out=rT_next[:, bsl], in_=pst)

            if stage < 99:
                nc.sync.dma_start(out=outT[t, 0:M, :], in_=rT_next)
                rT_prev = rT_next
                continue

            # output projection of this t is deferred into iteration t+1
            pending_out = (t, wo, h, rT_next, boc)
            rT_prev = rT_next

        if pending_out is not None and stage >= 99:
            emit_out(*pending_out)

    nc.compile()
    return nc


_CACHE = {}
LAST = {}


def _get_nc():
    if "nc" not in _CACHE:
        _CACHE["nc"] = build_nc()
    return _CACHE["nc"]


def host_prep(inputs, W1, b1, lng, lnb, W2, b2, Wih, Whh, bih, bhh,
              Wr, br, Ww, bw, Wo, bo, mem0, read0, wr0, ww0, h0, c0):
    f32 = np.float32
    inputs, W1, W2, Wih, Whh, Wr, Wo = [np.asarray(a, f32) for a in
                                        (inputs, W1, W2, Wih, Whh, Wr, Wo)]

    def percol(v, cols):   # [T, 128*cols] -> [T, 128, cols] column-major chunks
        return np.ascontiguousarray(
            np.asarray(v, f32).reshape(T, cols, 128).transpose(0, 2, 1))

    bf = ml_dtypes.bfloat16
    xT_full = np.ascontiguousarray(inputs.transpose(0, 2, 1)).astype(bf)
    w1t = np.ascontiguousarray(W1.transpose(0, 2, 1)).astype(bf)   # [T, E, H]
    w2t = np.ascontiguousarray(W2.transpose(0, 2, 1)).astype(bf)   # [T, H, V]
    wiht = np.ascontiguousarray(Wih.transpose(0, 2, 1)).astype(bf)
    whht = np.ascontiguousarray(Whh.transpose(0, 2, 1)).astype(bf)
    wrt = np.ascontiguousarray(Wr.transpose(0, 2, 1)).astype(bf)   # [T, H, 70]
    wot = np.ascontiguousarray(Wo.transpose(0, 2, 1)).astype(bf)   # [T, 576, E]
    h0t_full = np.asarray(h0, f32).transpose(0, 2, 1).astype(bf)
    c0t_full = np.asarray(c0, f32).transpose(0, 2, 1).astype(bf)
    r0t_full = np.asarray(read0, f32)[T - 1].T.astype(bf)          # [M, B]
    wr0_full = np.asarray(wr0, f32).astype(bf)
    mem0_full = np.asarray(mem0).astype(bf)
    bz = np.asarray(bih, f32) + np.asarray(bhh, f32)

    common = dict(
        w1t=w1t, w2t=w2t, wiht=wiht, whht=whht, wrt=wrt, wot=wot,
        b1c=percol(b1, HC), lngc=percol(lng, HC), lnbc=percol(lnb, HC),
        b2c=percol(b2, VC), bzc=percol(bz, ZC),
        brr=np.ascontiguousarray(np.asarray(br, f32).reshape(T, 1, M + 6)),
        boc=percol(bo, EC),
    )
    in_maps = []
    for ci in range(NCORES):
        bsl = slice(ci * BS, (ci + 1) * BS)
        in_maps.append(dict(
            common,
            xT=np.ascontiguousarray(xT_full[:, :, bsl]),
            h0t=np.ascontiguousarray(h0t_full[:, :, bsl]),
            c0t=np.ascontiguousarray(c0t_full[:, :, bsl]),
            r0t=np.ascontiguousarray(r0t_full[:, bsl]),
            wr0=np.ascontiguousarray(wr0_full[:, bsl, :]),
            mem0=np.ascontiguousarray(mem0_full[:, bsl]),
        ))

    return in_maps


def kernel(**inputs):
    in_maps = host_prep(**inputs)
    nc = _get_nc()
    import os
    trace = os.environ.get("BASS_TRACE", "") not in ("", "0")
    res = run_bass_kernel_spmd(nc, in_maps, list(range(NCORES)), trace=trace)
    LAST["exec_time_ns"] = res.exec_time_ns
    LAST["results"] = res
    out = np.concatenate(
        [np.transpose(r["outT"], (0, 2, 1)) for r in res.results], axis=1)
    return np.ascontiguousarray(out.astype(np.float32))
